# revision 2
# baseline (speedup 1.0000x reference)
"""Trainium2 Bass kernel for nn_CondRnnSampler — v3.

v2 (fp8 DoubleRow + all-tanh) was ACT-engine bound at ~9 ACTIVATE ops/step
(~1250 ns each, 95% busy).  v3 rebalances:
  - x = te[s]+pe gathered fully host-side (fp8 stream) — drops the te
    matmuls and the DVE x-adds.
  - esum/pick selector matmuls run DoubleRow over both k-halves (2 MMs/step
    instead of 4).
  - pick-product reads the logits PSUM directly on DVE (drops the ACT COPY).
  - relu+fp8-quant of the MLP hidden runs on GpSimd (Pool) instead of ACT.
  - one cell STT (x1) offloaded to GpSimd.
ACT per step: 4 gate tanh + state tanh + exp = 6 ops.

Per-core (512 rows), per step:
  MLP:   hid = relu(W1 h), logits = W2 hid, e = exp(logits), prod = logits*oh
  cell:  gates = W_ih x + W_hh h (fp8 DoubleRow, K=256/instr), all-tanh via
         sigma(z) = (1+tanh(z/2))/2.  State: s = 2c (bf16), v = 2h (fp8):
           s' = 0.5*(1+tf)*s + (1+ti)*g ;  v' = (1+to)*tanh(0.5 s')
  out:   esum/pick accumulate into [64,BS] PSUM banks via sliding-selector
         fp8 DoubleRow matmuls.

Scales (folded on host): x8 = 64*x, v = 2h, hid8 = 8*hid, gates PSUM = beta*a,
logits PSUM = delta*l.  One-hots (sample) and the full gathered LSTM input
(te[s]+pe) are built host-side and DMA-streamed.
"""

import sys

sys.path.insert(0, "/opt/trn_rl_repo")

from contextlib import ExitStack

import ml_dtypes
import numpy as np

import concourse.bacc as bacc
import concourse.tile as tile
from concourse import bass_utils, mybir
from concourse.bass import ts

B, D, E, NCL = 4096, 64, 256, 256
NCORES = 8
BS = B // NCORES
P = 128

AF = mybir.ActivationFunctionType
OP = mybir.AluOpType
F32 = mybir.dt.float32
BF16 = mybir.dt.bfloat16
FP8 = mybir.dt.float8e4
DR = mybir.MatmulPerfMode.DoubleRow
NPBF = ml_dtypes.bfloat16
NPF8 = ml_dtypes.float8_e4m3

SX = 64.0  # x fp8 scale
SH = 8.0  # hid fp8 scale
DELTA = 256.0  # logits PSUM scale


def _pe_table() -> np.ndarray:
    half = np.float32(E // 2)
    inv = (
        np.float32(1.0)
        / (np.float32(10000.0) ** (np.arange(E // 2, dtype=np.float32) / half))
    ).astype(np.float32)
    pos = np.arange(D, dtype=np.float32)[:, None]
    ang = pos * inv[None, :]
    return np.concatenate([np.sin(ang), np.cos(ang)], axis=1).astype(np.float32)


def _q8(x):
    return np.clip(np.asarray(x, np.float32), -240, 240).astype(NPF8)


def build_bass(n_steps: int = D):
    nc = bacc.Bacc("TRN2", debug=False, target_bir_lowering=False, num_devices=NCORES)

    def din(name, shape, dt):
        return nc.dram_tensor(name, list(shape), dt, kind="ExternalInput").ap()

    wih_d = din("wih", (P, 2, 4 * E), FP8)
    whh_d = din("whh", (P, 2, 4 * E), FP8)
    w1_d = din("w1", (P, 2, 2 * E), FP8)
    w2_d = din("w2", (P, 4, NCL), FP8)
    slide_d = din("slide", (P, 2, 2 * D), FP8)  # ones at col D-1 (both halves)
    ones64_d = din("ones64", (D, 1), F32)
    ohs_d = din("ohs", (D, P, 2, BS), FP8)  # one-hot(sample) per step
    xpe_d = din("xpe", (D, P, 2, BS), FP8)  # 64*(te[s]+petab[pos]) per step
    x0pe_d = din("x0pe", (P, 2, BS), FP8)  # 64*petab[pos_0] (init cell input)
    out_d = nc.dram_tensor("out", [1, BS], F32, kind="ExternalOutput").ap()

    with tile.TileContext(nc) as tc:
        with ExitStack() as ctx:
            sing = ctx.enter_context(tc.tile_pool(name="sing", bufs=1))
            gt = ctx.enter_context(tc.tile_pool(name="gt", bufs=6))
            gt2 = ctx.enter_context(tc.tile_pool(name="gt2", bufs=2))
            hp = ctx.enter_context(tc.tile_pool(name="hp", bufs=3))
            ep = ctx.enter_context(tc.tile_pool(name="ep", bufs=4))
            psing = ctx.enter_context(tc.tile_pool(name="psing", bufs=1, space="PSUM"))
            pp = ctx.enter_context(tc.tile_pool(name="pp", bufs=3, space="PSUM"))

            # ---- resident tensors -------------------------------------
            # init-critical first: step-0 one-hot/pe slices + gate weights
            ohs_sb = sing.tile([P, D, 2, BS], FP8, tag="ohs")
            xpe_sb = sing.tile([P, D, 2, BS], FP8, tag="xpe")
            x0pe_sb = sing.tile([P, 2, BS], FP8, tag="x0pe")
            nc.sync.dma_start(x0pe_sb[:], x0pe_d)
            nc.sync.dma_start(xpe_sb[:, 0], xpe_d[0])
            nc.sync.dma_start(ohs_sb[:, 0], ohs_d[0])
            wih = sing.tile([P, 2, 4 * E], FP8, tag="wih")
            nc.sync.dma_start(wih[:], wih_d)
            whh = sing.tile([P, 2, 4 * E], FP8, tag="whh")
            nc.sync.dma_start(whh[:], whh_d)
            w1 = sing.tile([P, 2, 2 * E], FP8, tag="w1")
            nc.sync.dma_start(w1[:], w1_d)
            w2 = sing.tile([P, 4, NCL], FP8, tag="w2")
            nc.sync.dma_start(w2[:], w2_d)
            slide = sing.tile([P, 2, 2 * D], FP8, tag="slide")
            nc.sync.dma_start(slide[:], slide_d)
            ones64 = sing.tile([D, 1], F32, tag="ones64")
            nc.sync.dma_start(ones64[:], ones64_d)

            for i in range(1, n_steps):
                nc.sync.dma_start(ohs_sb[:, i], ohs_d[i])
                nc.sync.dma_start(xpe_sb[:, i], xpe_d[i])

            # double-buffered recurrent state (parity by step)
            s_bufs = [
                sing.tile([P, 2, BS], BF16, tag=f"s{j}", name=f"s{j}")
                for j in range(2)
            ]
            v_bufs = [
                sing.tile([P, 2, BS], FP8, tag=f"v{j}", name=f"v{j}")
                for j in range(2)
            ]
            T_sb = sing.tile([P, 2, BS], BF16, tag="T")
            esum_ps = psing.tile([D, BS], F32, tag="esum")
            pick_ps = psing.tile([D, BS], F32, tag="pick")

            # scales arrive via sc tile? No - bake as python floats at build:
            # (they depend only on weight maxima; recomputed per call would
            # need rebuild. Instead scales are fixed: beta/gamma baked by
            # prep_inputs to match BETA/GAMMA globals.)

            def gate_step(x8_ap, v_prev, with_h, inv_beta, mid_cb=None):
                """gates -> t tiles [ti, tf, g, to]; order f,g,i,o so the
                chain ops X1 (needs tf) and X2 (needs g) unblock earliest.

                mid_cb (default-priority emissions) runs between the i- and
                o-gate blocks: the W1 matmuls land on PE right after whh-i,
                so the relus become ready (and clear DVE) well before the
                ti1/x2/s' chain junction needs the engine."""
                tg = [None] * 4
                # v-independent wih matmuls for the chain-leading f/g gates
                # are emitted at normal priority AHEAD of any whh matmul, so
                # the in-order PE queue runs them during the v-wait bubble
                # instead of stalling behind the first v-dependent whh.
                pre = {}
                if with_h:
                    for gi in (1, 2, 0):  # f, g, i
                        g_ps = pp.tile([P, 2, BS], F32, tag="ps")
                        for k in range(2):
                            nc.tensor.matmul(
                                g_ps[:, k, :], wih[:, :, ts(gi * 2 + k, P)],
                                x8_ap, start=True, stop=False, perf_mode=DR,
                            )
                        pre[gi] = g_ps
                with tc.high_priority():
                    for gi in (1, 2, 0):  # f, g, i
                        if gi in pre:
                            g_ps = pre[gi]
                            for k in range(2):
                                nc.tensor.matmul(
                                    g_ps[:, k, :], whh[:, :, ts(gi * 2 + k, P)],
                                    v_prev[:], start=False, stop=True,
                                    perf_mode=DR,
                                )
                        else:
                            g_ps = pp.tile([P, 2, BS], F32, tag="ps")
                            for k in range(2):
                                m = gi * 2 + k
                                nc.tensor.matmul(
                                    g_ps[:, k, :], wih[:, :, ts(m, P)], x8_ap,
                                    start=True, stop=not with_h, perf_mode=DR,
                                )
                                if with_h:
                                    nc.tensor.matmul(
                                        g_ps[:, k, :], whh[:, :, ts(m, P)],
                                        v_prev[:], start=False, stop=True,
                                        perf_mode=DR,
                                    )
                        t_sb = gt.tile([P, 2, BS], BF16, tag="t")
                        nc.scalar.activation(
                            t_sb[:], g_ps[:], AF.Tanh, scale=inv_beta
                        )
                        tg[gi] = t_sb
                if mid_cb is not None:
                    mid_cb()
                # o-gate last: its tanh is only needed by v' (after T)
                g_ps = pp.tile([P, 2, BS], F32, tag="ps")
                for k in range(2):
                    m = 3 * 2 + k
                    nc.tensor.matmul(
                        g_ps[:, k, :], wih[:, :, ts(m, P)], x8_ap,
                        start=True, stop=not with_h, perf_mode=DR,
                    )
                    if with_h:
                        nc.tensor.matmul(
                            g_ps[:, k, :], whh[:, :, ts(m, P)],
                            v_prev[:], start=False, stop=True, perf_mode=DR,
                        )
                t_sb = gt.tile([P, 2, BS], BF16, tag="t")
                nc.scalar.activation(t_sb[:], g_ps[:], AF.Tanh, scale=inv_beta)
                tg[3] = t_sb
                return tg

            def tail(tg, s_prev, s_cur, v_cur, first):
                """Recurrent-chain ops at high priority so the scheduler's
                static per-engine orders never park bulk work (relu/prod/
                exp) in front of them.

                Cell update decomposed into TS (4x bf16 mode) + TT (2x)
                ops instead of 1x-only STT:
                  sf  = 0.5 + 0.5*tf          (TS, 4x; folds the 0.5 of s')
                  x1  = sf * s_prev           (TT on GpSimd: tf/s_prev ready
                                               early, keeps DVE free)
                  ti1 = 1 + ti                (TS, 4x)
                  x2  = ti1 * g               (TT, 2x)
                  s'  = x1 + x2               (TT, 2x)
                  v'  = (1+to)*T              (STT: fp8 out is 1x anyway)
                """
                ti, tf, g, to = tg[0], tg[1], tg[2], tg[3]
                with tc.high_priority():
                    if first:
                        # s = (1+ti)*g
                        nc.vector.scalar_tensor_tensor(
                            s_cur[:], ti[:], 1.0, g[:], OP.add, OP.mult
                        )
                    else:
                        # sf = 0.5+0.5*tf (TS, 4x — folds s'-halving); x1 =
                        # sf*s (TT, 2x); x2 via STT (chain junction: one hop
                        # after tanh-i); s' = x1+x2 (TT, 2x)
                        sf = gt2.tile([P, 2, BS], BF16, tag="sf")
                        nc.vector.tensor_scalar(
                            sf[:], tf[:], 0.5, 0.5, OP.mult, OP.add
                        )
                        x1 = gt2.tile([P, 2, BS], BF16, tag="x1")
                        nc.gpsimd.tensor_tensor(
                            x1[:], sf[:], s_prev[:], OP.mult
                        )
                        x2 = gt2.tile([P, 2, BS], BF16, tag="x2")
                        nc.vector.scalar_tensor_tensor(
                            x2[:], ti[:], 1.0, g[:], OP.add, OP.mult
                        )
                        nc.vector.tensor_tensor(s_cur[:], x1[:], x2[:], OP.add)
                    nc.scalar.activation(T_sb[:], s_cur[:], AF.Tanh, scale=0.5)
                    nc.vector.scalar_tensor_tensor(
                        v_cur[:], to[:], 1.0, T_sb[:], OP.add, OP.mult
                    )

            inv_beta = float(1.0 / _BETA)
            hid_scale = float(SH / _GAMMA)
            inv_delta = float(1.0 / DELTA)

            # ---- init: lstm(pe_0) with zero state ---------------------
            # the init cell input is pe[pos[:,0]] alone (no token embed) so
            # it gets its own x0pe stream; scan step i uses xpe slot i =
            # te[s_i] + pe_i.
            tg0 = gate_step(x0pe_sb[:], None, with_h=False, inv_beta=inv_beta)
            tail(tg0, None, s_bufs[1], v_bufs[1], first=True)

            pending = []  # deferred (step, e8, pr8) awaiting esum/pick MMs

            def flush_accum(j, e8_j, pr8_j):
                nc.tensor.matmul(
                    esum_ps[:], slide[:, :, D - 1 - j : 2 * D - 1 - j],
                    e8_j[:], start=(j == 0), stop=(j == n_steps - 1),
                    perf_mode=DR, skip_group_check=True,
                )
                nc.tensor.matmul(
                    pick_ps[:], slide[:, :, D - 1 - j : 2 * D - 1 - j],
                    pr8_j[:], start=(j == 0), stop=(j == n_steps - 1),
                    perf_mode=DR, skip_group_check=True,
                )

            # ---- scan -------------------------------------------------
            for i in range(n_steps):
                v_prev, v_cur = v_bufs[(i + 1) % 2], v_bufs[i % 2]
                s_prev, s_cur = s_bufs[(i + 1) % 2], s_bufs[i % 2]

                hid8 = []

                def mlp_front(v_prev=v_prev, hid8=hid8):
                    # W1 + relus emitted mid-gate-block: PE runs W1 right
                    # after whh-i so both relus clear DVE before the chain
                    # junction (ti1/x2/s') needs it
                    for hh in range(2):
                        h_ps = pp.tile([P, 2, BS], F32, tag="ps")
                        for k in range(2):
                            m = hh * 2 + k
                            nc.tensor.matmul(
                                h_ps[:, k, :], w1[:, :, ts(m, P)], v_prev[:],
                                start=True, stop=True, perf_mode=DR,
                            )
                        h8 = hp.tile([P, 2, BS], FP8, tag="h8")
                        nc.vector.tensor_scalar(
                            h8[:], h_ps[:], hid_scale, 0.0, OP.mult, OP.max
                        )
                        hid8.append(h8)

                # gates + cell update FIRST (the serial chain)
                tg = gate_step(
                    xpe_sb[:, i], v_prev, with_h=True, inv_beta=inv_beta,
                    mid_cb=mlp_front,
                )
                tail(tg, s_prev, s_cur, v_cur, first=False)

                l_ps = pp.tile([P, 2, BS], F32, tag="ps")
                for t in range(2):
                    for j in range(2):
                        nc.tensor.matmul(
                            l_ps[:, t, :], w2[:, 2 * j : 2 * j + 2, ts(t, P)],
                            hid8[j][:], start=(j == 0), stop=(j == 1),
                            perf_mode=DR,
                        )
                e8 = ep.tile([P, 2, BS], FP8, tag="e8")
                nc.scalar.activation(e8[:], l_ps[:], AF.Exp, scale=inv_delta)
                # pick-product straight from the logits PSUM on DVE (one op;
                # the l_ps banks free after this + the exp read)
                pr8 = ep.tile([P, 2, BS], FP8, tag="pr8")
                nc.vector.tensor_tensor(
                    pr8[:], l_ps[:], ohs_sb[:, i], OP.mult
                )

                # esum/pick accumulation (fp8 DoubleRow; M=64 dst), deferred
                # by one step so these MMs never sit in the PE's in-order
                # queue ahead of the next step's chain-critical gate matmuls
                # while still waiting on exp/prod outputs.
                pending.append((i, e8, pr8))
                if i > 0:
                    flush_accum(*pending.pop(0))

            # ---- epilogue ---------------------------------------------
            while pending:
                flush_accum(*pending.pop(0))
            ln_e = sing.tile([D, BS], F32, tag="lne")
            nc.scalar.activation(ln_e[:], esum_ps[:], AF.Ln)
            diff = sing.tile([D, BS], F32, tag="diff")
            nc.vector.scalar_tensor_tensor(
                diff[:], pick_ps[:], inv_delta, ln_e[:],
                OP.mult, OP.subtract,
            )
            fin_ps = pp.tile([P, 2, BS], F32, tag="ps")
            nc.tensor.matmul(
                fin_ps[0:1, 0, :], ones64[:, 0:1], diff[:], start=True, stop=True
            )
            out_sb = sing.tile([1, BS], F32, tag="outsb")
            nc.scalar.activation(out_sb[:], fin_ps[0:1, 0, :], AF.Copy)
            nc.sync.dma_start(out_d, out_sb[:])

    nc.compile()
    return nc


_BETA = None
_GAMMA = None


def _compute_scales(W_ih, W_hh, W1):
    half = np.ones((4 * E, 1), np.float32)
    half[: 2 * E] = 0.5
    half[3 * E :] = 0.5
    Wg_ih = np.asarray(W_ih, np.float32) * half
    Wg_hh = np.asarray(W_hh, np.float32) * half
    beta = 216.0 / max(np.abs(Wg_ih / SX).max(), np.abs(Wg_hh / 2.0).max())
    gamma = 216.0 / np.abs(np.asarray(W1, np.float32) / 2.0).max()
    return beta, gamma, Wg_ih, Wg_hh


def prep_inputs(token_embed, W_ih, b_ih, b_hh, W_hh, W1, b1, W2, b2, pos_list,
                input_samples):
    f = np.float32
    for b in (b_ih, b_hh, b1, b2):
        assert np.all(np.asarray(b) == 0), "nonzero biases unsupported"
    beta, gamma, Wg_ih, Wg_hh = _compute_scales(W_ih, W_hh, W1)
    assert beta == _BETA and gamma == _GAMMA

    def lhsT8(Wt, ko):  # [K, M] -> [P, ko, M] fp8
        K, M = Wt.shape
        return np.ascontiguousarray(
            _q8(Wt).reshape(ko, P, M).transpose(1, 0, 2)
        )

    petab = _pe_table()
    slide = np.zeros((P, 2, 2 * D), f)
    slide[:, :, D - 1] = 1.0

    shared = {
        "wih": lhsT8(beta / SX * Wg_ih.T, 2),
        "whh": lhsT8(beta / 2.0 * Wg_hh.T, 2),
        "w1": lhsT8(gamma / 2.0 * np.asarray(W1, f).T, 2),
        "w2": lhsT8(DELTA / SH * np.asarray(W2, f).T, 4),
        "slide": _q8(slide),
        "ones64": np.ones((D, 1), f),
    }
    samples = np.asarray(input_samples)
    poss = np.asarray(pos_list)
    te_f = np.asarray(token_embed, f)  # [NCL, E]
    in_maps = []
    for c in range(NCORES):
        lo, hi = c * BS, (c + 1) * BS
        sa = samples[lo:hi]  # [BS, D]
        po = poss[lo:hi]
        ohs = np.zeros((D, 2, P, BS), NPF8)
        ii = np.arange(BS)
        for i in range(D):
            s = np.asarray(sa[:, i])
            ohs[i, s // P, s % P, ii] = 1.0
        ohs = np.ascontiguousarray(ohs.transpose(0, 2, 1, 3))
        # full LSTM input per step: x_i = te[s_i] + pe(pos_i), fp8 at 64x
        xpe = _q8(SX * (te_f[sa.T] + petab[po.T]))  # [D, BS, E]
        xpe = np.ascontiguousarray(
            xpe.transpose(0, 2, 1).reshape(D, 2, P, BS).transpose(0, 2, 1, 3)
        )
        x0pe = _q8(SX * petab[po[:, 0]])  # [BS, E] — init cell input (pe only)
        x0pe = np.ascontiguousarray(
            x0pe.T.reshape(2, P, BS).transpose(1, 0, 2)
        )
        m = dict(shared)
        m["ohs"] = ohs
        m["xpe"] = xpe
        m["x0pe"] = x0pe
        in_maps.append(m)
    return in_maps


_CACHE = {}


def kernel(**inputs) -> np.ndarray:
    global _BETA, _GAMMA
    if "nc" not in _CACHE:
        _BETA, _GAMMA, _, _ = _compute_scales(
            inputs["W_ih"], inputs["W_hh"], inputs["W1"]
        )
        _CACHE["nc"] = build_bass()
    nc = _CACHE["nc"]
    in_maps = prep_inputs(**inputs)
    res = bass_utils.run_bass_kernel_spmd(nc, in_maps, core_ids=list(range(NCORES)))
    _CACHE["last_results"] = res
    out = np.empty((B, 1), np.float32)
    for c in range(NCORES):
        out[c * BS : (c + 1) * BS, 0] = np.asarray(
            res.results[c]["out"], np.float32
        ).reshape(BS)
    return out



# revision 3
# speedup vs baseline: 1.0316x; 1.0316x over previous
"""Trainium2 Bass kernel for nn_CondRnnSampler — v3.

v2 (fp8 DoubleRow + all-tanh) was ACT-engine bound at ~9 ACTIVATE ops/step
(~1250 ns each, 95% busy); after rebalancing the kernel is chain-latency
bound (~9.9 us/step), so emission order targets the recurrence path.
v3 changes:
  - x = te[s]+pe gathered fully host-side (fp8 stream) — drops the te
    matmuls and the DVE x-adds.
  - esum/pick selector matmuls run DoubleRow over both k-halves (2 MMs/step
    instead of 4).
  - pick-product reads the logits PSUM directly on DVE (drops the ACT COPY).
  - both MLP relus on DVE, emitted via gate_step's mid_cb so their W1
    matmuls run right after whh-i on PE and the relus clear DVE before the
    chain junction (x2/s') needs it; the o-gate block comes after.
  - cell update as TS (4x) + TT (2x) + STT mix; x1 = sf*s_prev runs on
    GpSimd (off the critical path — the junction is the x2 path).
ACT per step: 4 gate tanh + state tanh + exp = 6 ops.

Per-core (512 rows), per step:
  MLP:   hid = relu(W1 h), logits = W2 hid, e = exp(logits), prod = logits*oh
  cell:  gates = W_ih x + W_hh h (fp8 DoubleRow, K=256/instr), all-tanh via
         sigma(z) = (1+tanh(z/2))/2.  State: s = 2c (bf16), v = 2h (fp8):
           s' = 0.5*(1+tf)*s + (1+ti)*g ;  v' = (1+to)*tanh(0.5 s')
  out:   esum/pick accumulate into [64,BS] PSUM banks via sliding-selector
         fp8 DoubleRow matmuls.

Scales (folded on host): x8 = 64*x, v = 2h, hid8 = 8*hid, gates PSUM = beta*a,
logits PSUM = delta*l.  One-hots (sample) and the full gathered LSTM input
(te[s]+pe) are built host-side and DMA-streamed.
"""

import sys

sys.path.insert(0, "/opt/trn_rl_repo")

from contextlib import ExitStack

import ml_dtypes
import numpy as np

import concourse.bacc as bacc
import concourse.tile as tile
from concourse import bass_utils, mybir
from concourse.bass import ts

B, D, E, NCL = 4096, 64, 256, 256
NCORES = 8
BS = B // NCORES
P = 128

AF = mybir.ActivationFunctionType
OP = mybir.AluOpType
F32 = mybir.dt.float32
BF16 = mybir.dt.bfloat16
FP8 = mybir.dt.float8e4
DR = mybir.MatmulPerfMode.DoubleRow
NPBF = ml_dtypes.bfloat16
NPF8 = ml_dtypes.float8_e4m3

SX = 64.0  # x fp8 scale
SH = 8.0  # hid fp8 scale
DELTA = 256.0  # logits PSUM scale


def _pe_table() -> np.ndarray:
    half = np.float32(E // 2)
    inv = (
        np.float32(1.0)
        / (np.float32(10000.0) ** (np.arange(E // 2, dtype=np.float32) / half))
    ).astype(np.float32)
    pos = np.arange(D, dtype=np.float32)[:, None]
    ang = pos * inv[None, :]
    return np.concatenate([np.sin(ang), np.cos(ang)], axis=1).astype(np.float32)


def _q8(x):
    return np.clip(np.asarray(x, np.float32), -240, 240).astype(NPF8)


def build_bass(n_steps: int = D):
    nc = bacc.Bacc("TRN2", debug=False, target_bir_lowering=False, num_devices=NCORES)

    def din(name, shape, dt):
        return nc.dram_tensor(name, list(shape), dt, kind="ExternalInput").ap()

    wih_d = din("wih", (P, 2, 4 * E), FP8)
    whh_d = din("whh", (P, 2, 4 * E), FP8)
    w1_d = din("w1", (P, 2, 2 * E), FP8)
    w2_d = din("w2", (P, 4, NCL), FP8)
    slide_d = din("slide", (P, 2, 2 * D), FP8)  # ones at col D-1 (both halves)
    ones64_d = din("ones64", (D, 1), F32)
    ohs_d = din("ohs", (D, P, 2, BS), FP8)  # one-hot(sample) per step
    xpe_d = din("xpe", (D, P, 2, BS), FP8)  # 64*(te[s]+petab[pos]) per step
    x0pe_d = din("x0pe", (P, 2, BS), FP8)  # 64*petab[pos_0] (init cell input)
    out_d = nc.dram_tensor("out", [1, BS], F32, kind="ExternalOutput").ap()

    with tile.TileContext(nc) as tc:
        with ExitStack() as ctx:
            sing = ctx.enter_context(tc.tile_pool(name="sing", bufs=1))
            gt = ctx.enter_context(tc.tile_pool(name="gt", bufs=6))
            gt2 = ctx.enter_context(tc.tile_pool(name="gt2", bufs=2))
            hp = ctx.enter_context(tc.tile_pool(name="hp", bufs=3))
            ep = ctx.enter_context(tc.tile_pool(name="ep", bufs=4))
            psing = ctx.enter_context(tc.tile_pool(name="psing", bufs=1, space="PSUM"))
            pp = ctx.enter_context(tc.tile_pool(name="pp", bufs=3, space="PSUM"))

            # ---- resident tensors -------------------------------------
            # init-critical first: step-0 one-hot/pe slices + gate weights
            ohs_sb = sing.tile([P, D, 2, BS], FP8, tag="ohs")
            xpe_sb = sing.tile([P, D, 2, BS], FP8, tag="xpe")
            x0pe_sb = sing.tile([P, 2, BS], FP8, tag="x0pe")
            nc.sync.dma_start(x0pe_sb[:], x0pe_d)
            nc.sync.dma_start(xpe_sb[:, 0], xpe_d[0])
            nc.sync.dma_start(ohs_sb[:, 0], ohs_d[0])
            wih = sing.tile([P, 2, 4 * E], FP8, tag="wih")
            nc.sync.dma_start(wih[:], wih_d)
            whh = sing.tile([P, 2, 4 * E], FP8, tag="whh")
            nc.sync.dma_start(whh[:], whh_d)
            w1 = sing.tile([P, 2, 2 * E], FP8, tag="w1")
            nc.sync.dma_start(w1[:], w1_d)
            w2 = sing.tile([P, 4, NCL], FP8, tag="w2")
            nc.sync.dma_start(w2[:], w2_d)
            slide = sing.tile([P, 2, 2 * D], FP8, tag="slide")
            nc.sync.dma_start(slide[:], slide_d)
            ones64 = sing.tile([D, 1], F32, tag="ones64")
            nc.sync.dma_start(ones64[:], ones64_d)

            for i in range(1, n_steps):
                nc.sync.dma_start(ohs_sb[:, i], ohs_d[i])
                nc.sync.dma_start(xpe_sb[:, i], xpe_d[i])

            # double-buffered recurrent state (parity by step)
            s_bufs = [
                sing.tile([P, 2, BS], BF16, tag=f"s{j}", name=f"s{j}")
                for j in range(2)
            ]
            v_bufs = [
                sing.tile([P, 2, BS], FP8, tag=f"v{j}", name=f"v{j}")
                for j in range(2)
            ]
            T_sb = sing.tile([P, 2, BS], BF16, tag="T")
            esum_ps = psing.tile([D, BS], F32, tag="esum")
            pick_ps = psing.tile([D, BS], F32, tag="pick")

            # scales arrive via sc tile? No - bake as python floats at build:
            # (they depend only on weight maxima; recomputed per call would
            # need rebuild. Instead scales are fixed: beta/gamma baked by
            # prep_inputs to match BETA/GAMMA globals.)

            def gate_step(x8_ap, v_prev, with_h, inv_beta, mid_cb=None):
                """gates -> t tiles [ti, tf, g, to]; order f,g,i,o so the
                chain ops X1 (needs tf) and X2 (needs g) unblock earliest.

                mid_cb (default-priority emissions) runs between the i- and
                o-gate blocks: the W1 matmuls land on PE right after whh-i,
                so the relus become ready (and clear DVE) well before the
                ti1/x2/s' chain junction needs the engine."""
                tg = [None] * 4
                # v-independent wih matmuls for the chain-leading f/g gates
                # are emitted at normal priority AHEAD of any whh matmul, so
                # the in-order PE queue runs them during the v-wait bubble
                # instead of stalling behind the first v-dependent whh.
                pre = {}
                if with_h:
                    for gi in (1, 2, 0):  # f, g, i
                        g_ps = pp.tile([P, 2, BS], F32, tag="ps")
                        for k in range(2):
                            nc.tensor.matmul(
                                g_ps[:, k, :], wih[:, :, ts(gi * 2 + k, P)],
                                x8_ap, start=True, stop=False, perf_mode=DR,
                            )
                        pre[gi] = g_ps
                with tc.high_priority():
                    for gi in (1, 2, 0):  # f, g, i
                        if gi in pre:
                            g_ps = pre[gi]
                            for k in range(2):
                                nc.tensor.matmul(
                                    g_ps[:, k, :], whh[:, :, ts(gi * 2 + k, P)],
                                    v_prev[:], start=False, stop=True,
                                    perf_mode=DR,
                                )
                        else:
                            g_ps = pp.tile([P, 2, BS], F32, tag="ps")
                            for k in range(2):
                                m = gi * 2 + k
                                nc.tensor.matmul(
                                    g_ps[:, k, :], wih[:, :, ts(m, P)], x8_ap,
                                    start=True, stop=not with_h, perf_mode=DR,
                                )
                                if with_h:
                                    nc.tensor.matmul(
                                        g_ps[:, k, :], whh[:, :, ts(m, P)],
                                        v_prev[:], start=False, stop=True,
                                        perf_mode=DR,
                                    )
                        t_sb = gt.tile([P, 2, BS], BF16, tag="t")
                        nc.scalar.activation(
                            t_sb[:], g_ps[:], AF.Tanh, scale=inv_beta
                        )
                        tg[gi] = t_sb
                if mid_cb is not None:
                    mid_cb()
                # o-gate last: its tanh is only needed by v' (after T)
                g_ps = pp.tile([P, 2, BS], F32, tag="ps")
                for k in range(2):
                    m = 3 * 2 + k
                    nc.tensor.matmul(
                        g_ps[:, k, :], wih[:, :, ts(m, P)], x8_ap,
                        start=True, stop=not with_h, perf_mode=DR,
                    )
                    if with_h:
                        nc.tensor.matmul(
                            g_ps[:, k, :], whh[:, :, ts(m, P)],
                            v_prev[:], start=False, stop=True, perf_mode=DR,
                        )
                t_sb = gt.tile([P, 2, BS], BF16, tag="t")
                nc.scalar.activation(t_sb[:], g_ps[:], AF.Tanh, scale=inv_beta)
                tg[3] = t_sb
                return tg

            def tail(tg, s_prev, s_cur, v_cur, first):
                """Recurrent-chain ops at high priority so the scheduler's
                static per-engine orders never park bulk work (relu/prod/
                exp) in front of them.

                Cell update decomposed into TS (4x bf16 mode) + TT (2x)
                ops instead of 1x-only STT:
                  sf  = 0.5 + 0.5*tf          (TS, 4x; folds the 0.5 of s')
                  x1  = sf * s_prev           (TT on GpSimd: tf/s_prev ready
                                               early, keeps DVE free)
                  ti1 = 1 + ti                (TS, 4x)
                  x2  = ti1 * g               (TT, 2x)
                  s'  = x1 + x2               (TT, 2x)
                  v'  = (1+to)*T              (STT: fp8 out is 1x anyway)
                """
                ti, tf, g, to = tg[0], tg[1], tg[2], tg[3]
                with tc.high_priority():
                    if first:
                        # s = (1+ti)*g
                        nc.vector.scalar_tensor_tensor(
                            s_cur[:], ti[:], 1.0, g[:], OP.add, OP.mult
                        )
                    else:
                        # sf = 0.5+0.5*tf (TS, 4x — folds s'-halving); x1 =
                        # sf*s (TT, 2x); x2 via STT (chain junction: one hop
                        # after tanh-i); s' = x1+x2 (TT, 2x)
                        sf = gt2.tile([P, 2, BS], BF16, tag="sf")
                        nc.vector.tensor_scalar(
                            sf[:], tf[:], 0.5, 0.5, OP.mult, OP.add
                        )
                        x1 = gt2.tile([P, 2, BS], BF16, tag="x1")
                        nc.gpsimd.tensor_tensor(
                            x1[:], sf[:], s_prev[:], OP.mult
                        )
                        x2 = gt2.tile([P, 2, BS], BF16, tag="x2")
                        nc.vector.scalar_tensor_tensor(
                            x2[:], ti[:], 1.0, g[:], OP.add, OP.mult
                        )
                        nc.vector.tensor_tensor(s_cur[:], x1[:], x2[:], OP.add)
                    nc.scalar.activation(T_sb[:], s_cur[:], AF.Tanh, scale=0.5)
                    nc.vector.scalar_tensor_tensor(
                        v_cur[:], to[:], 1.0, T_sb[:], OP.add, OP.mult
                    )

            inv_beta = float(1.0 / _BETA)
            hid_scale = float(SH / _GAMMA)
            inv_delta = float(1.0 / DELTA)

            # ---- init: lstm(pe_0) with zero state ---------------------
            # the init cell input is pe[pos[:,0]] alone (no token embed) so
            # it gets its own x0pe stream; scan step i uses xpe slot i =
            # te[s_i] + pe_i.
            tg0 = gate_step(x0pe_sb[:], None, with_h=False, inv_beta=inv_beta)
            tail(tg0, None, s_bufs[1], v_bufs[1], first=True)

            pending = []  # deferred (step, e8, pr8) awaiting esum/pick MMs

            def flush_accum(j, e8_j, pr8_j):
                nc.tensor.matmul(
                    esum_ps[:], slide[:, :, D - 1 - j : 2 * D - 1 - j],
                    e8_j[:], start=(j == 0), stop=(j == n_steps - 1),
                    perf_mode=DR, skip_group_check=True,
                )
                nc.tensor.matmul(
                    pick_ps[:], slide[:, :, D - 1 - j : 2 * D - 1 - j],
                    pr8_j[:], start=(j == 0), stop=(j == n_steps - 1),
                    perf_mode=DR, skip_group_check=True,
                )

            # ---- scan -------------------------------------------------
            for i in range(n_steps):
                v_prev, v_cur = v_bufs[(i + 1) % 2], v_bufs[i % 2]
                s_prev, s_cur = s_bufs[(i + 1) % 2], s_bufs[i % 2]

                hid8 = []

                def mlp_front(v_prev=v_prev, hid8=hid8):
                    # W1 + relus emitted mid-gate-block: PE runs W1 right
                    # after whh-i so both relus clear DVE before the chain
                    # junction (ti1/x2/s') needs it
                    for hh in range(2):
                        h_ps = pp.tile([P, 2, BS], F32, tag="ps")
                        for k in range(2):
                            m = hh * 2 + k
                            nc.tensor.matmul(
                                h_ps[:, k, :], w1[:, :, ts(m, P)], v_prev[:],
                                start=True, stop=True, perf_mode=DR,
                            )
                        h8 = hp.tile([P, 2, BS], FP8, tag="h8")
                        nc.vector.tensor_scalar(
                            h8[:], h_ps[:], hid_scale, 0.0, OP.mult, OP.max
                        )
                        hid8.append(h8)

                # gates + cell update FIRST (the serial chain)
                tg = gate_step(
                    xpe_sb[:, i], v_prev, with_h=True, inv_beta=inv_beta,
                    mid_cb=mlp_front,
                )
                tail(tg, s_prev, s_cur, v_cur, first=False)

                l_ps = pp.tile([P, 2, BS], F32, tag="ps")
                for t in range(2):
                    for j in range(2):
                        nc.tensor.matmul(
                            l_ps[:, t, :], w2[:, 2 * j : 2 * j + 2, ts(t, P)],
                            hid8[j][:], start=(j == 0), stop=(j == 1),
                            perf_mode=DR,
                        )
                e8 = ep.tile([P, 2, BS], FP8, tag="e8")
                nc.scalar.activation(e8[:], l_ps[:], AF.Exp, scale=inv_delta)
                # pick-product straight from the logits PSUM on DVE (one op;
                # the l_ps banks free after this + the exp read)
                pr8 = ep.tile([P, 2, BS], FP8, tag="pr8")
                nc.vector.tensor_tensor(
                    pr8[:], l_ps[:], ohs_sb[:, i], OP.mult
                )

                # esum/pick accumulation (fp8 DoubleRow; M=64 dst), deferred
                # by one step so these MMs never sit in the PE's in-order
                # queue ahead of the next step's chain-critical gate matmuls
                # while still waiting on exp/prod outputs.
                pending.append((i, e8, pr8))
                if i > 0:
                    flush_accum(*pending.pop(0))

            # ---- epilogue ---------------------------------------------
            while pending:
                flush_accum(*pending.pop(0))
            ln_e = sing.tile([D, BS], F32, tag="lne")
            nc.scalar.activation(ln_e[:], esum_ps[:], AF.Ln)
            diff = sing.tile([D, BS], F32, tag="diff")
            nc.vector.scalar_tensor_tensor(
                diff[:], pick_ps[:], inv_delta, ln_e[:],
                OP.mult, OP.subtract,
            )
            fin_ps = pp.tile([P, 2, BS], F32, tag="ps")
            nc.tensor.matmul(
                fin_ps[0:1, 0, :], ones64[:, 0:1], diff[:], start=True, stop=True
            )
            out_sb = sing.tile([1, BS], F32, tag="outsb")
            nc.scalar.activation(out_sb[:], fin_ps[0:1, 0, :], AF.Copy)
            nc.sync.dma_start(out_d, out_sb[:])

    nc.compile()
    return nc


_BETA = None
_GAMMA = None


def _compute_scales(W_ih, W_hh, W1):
    half = np.ones((4 * E, 1), np.float32)
    half[: 2 * E] = 0.5
    half[3 * E :] = 0.5
    Wg_ih = np.asarray(W_ih, np.float32) * half
    Wg_hh = np.asarray(W_hh, np.float32) * half
    beta = 216.0 / max(np.abs(Wg_ih / SX).max(), np.abs(Wg_hh / 2.0).max())
    gamma = 216.0 / np.abs(np.asarray(W1, np.float32) / 2.0).max()
    return beta, gamma, Wg_ih, Wg_hh


def prep_inputs(token_embed, W_ih, b_ih, b_hh, W_hh, W1, b1, W2, b2, pos_list,
                input_samples):
    f = np.float32
    for b in (b_ih, b_hh, b1, b2):
        assert np.all(np.asarray(b) == 0), "nonzero biases unsupported"
    beta, gamma, Wg_ih, Wg_hh = _compute_scales(W_ih, W_hh, W1)
    assert beta == _BETA and gamma == _GAMMA

    def lhsT8(Wt, ko):  # [K, M] -> [P, ko, M] fp8
        K, M = Wt.shape
        return np.ascontiguousarray(
            _q8(Wt).reshape(ko, P, M).transpose(1, 0, 2)
        )

    petab = _pe_table()
    slide = np.zeros((P, 2, 2 * D), f)
    slide[:, :, D - 1] = 1.0

    shared = {
        "wih": lhsT8(beta / SX * Wg_ih.T, 2),
        "whh": lhsT8(beta / 2.0 * Wg_hh.T, 2),
        "w1": lhsT8(gamma / 2.0 * np.asarray(W1, f).T, 2),
        "w2": lhsT8(DELTA / SH * np.asarray(W2, f).T, 4),
        "slide": _q8(slide),
        "ones64": np.ones((D, 1), f),
    }
    samples = np.asarray(input_samples)
    poss = np.asarray(pos_list)
    te_f = np.asarray(token_embed, f)  # [NCL, E]
    in_maps = []
    for c in range(NCORES):
        lo, hi = c * BS, (c + 1) * BS
        sa = samples[lo:hi]  # [BS, D]
        po = poss[lo:hi]
        ohs = np.zeros((D, 2, P, BS), NPF8)
        ii = np.arange(BS)
        for i in range(D):
            s = np.asarray(sa[:, i])
            ohs[i, s // P, s % P, ii] = 1.0
        ohs = np.ascontiguousarray(ohs.transpose(0, 2, 1, 3))
        # full LSTM input per step: x_i = te[s_i] + pe(pos_i), fp8 at 64x
        xpe = _q8(SX * (te_f[sa.T] + petab[po.T]))  # [D, BS, E]
        xpe = np.ascontiguousarray(
            xpe.transpose(0, 2, 1).reshape(D, 2, P, BS).transpose(0, 2, 1, 3)
        )
        x0pe = _q8(SX * petab[po[:, 0]])  # [BS, E] — init cell input (pe only)
        x0pe = np.ascontiguousarray(
            x0pe.T.reshape(2, P, BS).transpose(1, 0, 2)
        )
        m = dict(shared)
        m["ohs"] = ohs
        m["xpe"] = xpe
        m["x0pe"] = x0pe
        in_maps.append(m)
    return in_maps


_CACHE = {}


def kernel(**inputs) -> np.ndarray:
    global _BETA, _GAMMA
    if "nc" not in _CACHE:
        _BETA, _GAMMA, _, _ = _compute_scales(
            inputs["W_ih"], inputs["W_hh"], inputs["W1"]
        )
        _CACHE["nc"] = build_bass()
    nc = _CACHE["nc"]
    in_maps = prep_inputs(**inputs)
    res = bass_utils.run_bass_kernel_spmd(nc, in_maps, core_ids=list(range(NCORES)))
    _CACHE["last_results"] = res
    out = np.empty((B, 1), np.float32)
    for c in range(NCORES):
        out[c * BS : (c + 1) * BS, 0] = np.asarray(
            res.results[c]["out"], np.float32
        ).reshape(BS)
    return out



# revision 4
# speedup vs baseline: 1.0324x; 1.0008x over previous
"""Trainium2 Bass kernel for nn_CondRnnSampler — v3.

v2 (fp8 DoubleRow + all-tanh) was ACT-engine bound at ~9 ACTIVATE ops/step
(~1250 ns each, 95% busy); after rebalancing the kernel is chain-latency
bound (~9.9 us/step), so emission order targets the recurrence path.
v3 changes:
  - x = te[s]+pe gathered fully host-side (fp8 stream) — drops the te
    matmuls and the DVE x-adds.
  - esum/pick selector matmuls run DoubleRow over both k-halves (2 MMs/step
    instead of 4).
  - pick-product reads the logits PSUM directly on DVE (drops the ACT COPY).
  - both MLP relus on DVE, emitted via gate_step's mid_cb so their W1
    matmuls run right after whh-i on PE and the relus clear DVE before the
    chain junction (x2/s') needs it; the o-gate block comes after.
  - cell update as TS (4x) + TT (2x) + STT mix; x1 = sf*s_prev runs on
    GpSimd (off the critical path — the junction is the x2 path).
ACT per step: 4 gate tanh + state tanh + exp = 6 ops.

Per-core (512 rows), per step:
  MLP:   hid = relu(W1 h), logits = W2 hid, e = exp(logits), prod = logits*oh
  cell:  gates = W_ih x + W_hh h (fp8 DoubleRow, K=256/instr), all-tanh via
         sigma(z) = (1+tanh(z/2))/2.  State: s = 2c (bf16), v = 2h (fp8):
           s' = 0.5*(1+tf)*s + (1+ti)*g ;  v' = (1+to)*tanh(0.5 s')
  out:   esum/pick accumulate into [64,BS] PSUM banks via sliding-selector
         fp8 DoubleRow matmuls.

Scales (folded on host): x8 = 64*x, v = 2h, hid8 = 8*hid, gates PSUM = beta*a,
logits PSUM = delta*l.  One-hots (sample) and the full gathered LSTM input
(te[s]+pe) are built host-side and DMA-streamed.
"""

import sys

sys.path.insert(0, "/opt/trn_rl_repo")

from contextlib import ExitStack

import ml_dtypes
import numpy as np

import concourse.bacc as bacc
import concourse.tile as tile
from concourse import bass_utils, mybir
from concourse.bass import ts

B, D, E, NCL = 4096, 64, 256, 256
NCORES = 8
BS = B // NCORES
P = 128

AF = mybir.ActivationFunctionType
OP = mybir.AluOpType
F32 = mybir.dt.float32
BF16 = mybir.dt.bfloat16
FP8 = mybir.dt.float8e4
DR = mybir.MatmulPerfMode.DoubleRow
NPBF = ml_dtypes.bfloat16
NPF8 = ml_dtypes.float8_e4m3

SX = 64.0  # x fp8 scale
SH = 8.0  # hid fp8 scale
DELTA = 256.0  # logits PSUM scale


def _pe_table() -> np.ndarray:
    half = np.float32(E // 2)
    inv = (
        np.float32(1.0)
        / (np.float32(10000.0) ** (np.arange(E // 2, dtype=np.float32) / half))
    ).astype(np.float32)
    pos = np.arange(D, dtype=np.float32)[:, None]
    ang = pos * inv[None, :]
    return np.concatenate([np.sin(ang), np.cos(ang)], axis=1).astype(np.float32)


def _q8(x):
    return np.clip(np.asarray(x, np.float32), -240, 240).astype(NPF8)


def build_bass(n_steps: int = D):
    nc = bacc.Bacc("TRN2", debug=False, target_bir_lowering=False, num_devices=NCORES)

    def din(name, shape, dt):
        return nc.dram_tensor(name, list(shape), dt, kind="ExternalInput").ap()

    wih_d = din("wih", (P, 2, 4 * E), FP8)
    whh_d = din("whh", (P, 2, 4 * E), FP8)
    w1_d = din("w1", (P, 2, 2 * E), FP8)
    w2_d = din("w2", (P, 4, NCL), FP8)
    slide_d = din("slide", (P, 2, 2 * D), FP8)  # ones at col D-1 (both halves)
    ones64_d = din("ones64", (D, 1), F32)
    ohs_d = din("ohs", (D, P, 2, BS), FP8)  # one-hot(sample) per step
    xpe_d = din("xpe", (D, P, 2, BS), FP8)  # 64*(te[s]+petab[pos]) per step
    x0pe_d = din("x0pe", (P, 2, BS), FP8)  # 64*petab[pos_0] (init cell input)
    out_d = nc.dram_tensor("out", [1, BS], F32, kind="ExternalOutput").ap()

    with tile.TileContext(nc) as tc:
        with ExitStack() as ctx:
            sing = ctx.enter_context(tc.tile_pool(name="sing", bufs=1))
            gt = ctx.enter_context(tc.tile_pool(name="gt", bufs=6))
            gt2 = ctx.enter_context(tc.tile_pool(name="gt2", bufs=2))
            hp = ctx.enter_context(tc.tile_pool(name="hp", bufs=3))
            ep = ctx.enter_context(tc.tile_pool(name="ep", bufs=4))
            psing = ctx.enter_context(tc.tile_pool(name="psing", bufs=1, space="PSUM"))
            pp = ctx.enter_context(tc.tile_pool(name="pp", bufs=3, space="PSUM"))

            # ---- resident tensors -------------------------------------
            # init-critical first: step-0 one-hot/pe slices + gate weights
            ohs_sb = sing.tile([P, D, 2, BS], FP8, tag="ohs")
            xpe_sb = sing.tile([P, D, 2, BS], FP8, tag="xpe")
            x0pe_sb = sing.tile([P, 2, BS], FP8, tag="x0pe")
            nc.sync.dma_start(x0pe_sb[:], x0pe_d)
            nc.sync.dma_start(xpe_sb[:, 0], xpe_d[0])
            nc.sync.dma_start(ohs_sb[:, 0], ohs_d[0])
            wih = sing.tile([P, 2, 4 * E], FP8, tag="wih")
            nc.sync.dma_start(wih[:], wih_d)
            whh = sing.tile([P, 2, 4 * E], FP8, tag="whh")
            nc.sync.dma_start(whh[:], whh_d)
            w1 = sing.tile([P, 2, 2 * E], FP8, tag="w1")
            nc.sync.dma_start(w1[:], w1_d)
            w2 = sing.tile([P, 4, NCL], FP8, tag="w2")
            nc.sync.dma_start(w2[:], w2_d)
            slide = sing.tile([P, 2, 2 * D], FP8, tag="slide")
            nc.sync.dma_start(slide[:], slide_d)
            ones64 = sing.tile([D, 1], F32, tag="ones64")
            nc.sync.dma_start(ones64[:], ones64_d)

            for i in range(1, n_steps):
                nc.sync.dma_start(ohs_sb[:, i], ohs_d[i])
                nc.sync.dma_start(xpe_sb[:, i], xpe_d[i])

            # double-buffered recurrent state (parity by step)
            s_bufs = [
                sing.tile([P, 2, BS], BF16, tag=f"s{j}", name=f"s{j}")
                for j in range(2)
            ]
            v_bufs = [
                sing.tile([P, 2, BS], FP8, tag=f"v{j}", name=f"v{j}")
                for j in range(2)
            ]
            T_sb = sing.tile([P, 2, BS], BF16, tag="T")
            esum_ps = psing.tile([D, BS], F32, tag="esum")
            pick_ps = psing.tile([D, BS], F32, tag="pick")

            # scales arrive via sc tile? No - bake as python floats at build:
            # (they depend only on weight maxima; recomputed per call would
            # need rebuild. Instead scales are fixed: beta/gamma baked by
            # prep_inputs to match BETA/GAMMA globals.)

            def gate_step(x8_ap, v_prev, with_h, inv_beta, mid_cb=None):
                """gates -> t tiles [ti, tf, g, to]; order f,g,i,o so the
                chain ops X1 (needs tf) and X2 (needs g) unblock earliest.

                mid_cb (default-priority emissions) runs between the i- and
                o-gate blocks: the W1 matmuls land on PE right after whh-i,
                so the relus become ready (and clear DVE) well before the
                ti1/x2/s' chain junction needs the engine."""
                tg = [None] * 4
                # v-independent wih matmuls for the chain-leading f/g gates
                # are emitted at normal priority AHEAD of any whh matmul, so
                # the in-order PE queue runs them during the v-wait bubble
                # instead of stalling behind the first v-dependent whh.
                pre = {}
                if with_h:
                    for gi in (1, 2, 0):  # f, g, i
                        g_ps = pp.tile([P, 2, BS], F32, tag="ps")
                        for k in range(2):
                            nc.tensor.matmul(
                                g_ps[:, k, :], wih[:, :, ts(gi * 2 + k, P)],
                                x8_ap, start=True, stop=False, perf_mode=DR,
                            )
                        pre[gi] = g_ps
                with tc.high_priority():
                    for gi in (1, 2, 0):  # f, g, i
                        if gi in pre:
                            g_ps = pre[gi]
                            for k in range(2):
                                nc.tensor.matmul(
                                    g_ps[:, k, :], whh[:, :, ts(gi * 2 + k, P)],
                                    v_prev[:], start=False, stop=True,
                                    perf_mode=DR,
                                )
                        else:
                            g_ps = pp.tile([P, 2, BS], F32, tag="ps")
                            for k in range(2):
                                m = gi * 2 + k
                                nc.tensor.matmul(
                                    g_ps[:, k, :], wih[:, :, ts(m, P)], x8_ap,
                                    start=True, stop=not with_h, perf_mode=DR,
                                )
                                if with_h:
                                    nc.tensor.matmul(
                                        g_ps[:, k, :], whh[:, :, ts(m, P)],
                                        v_prev[:], start=False, stop=True,
                                        perf_mode=DR,
                                    )
                        t_sb = gt.tile([P, 2, BS], BF16, tag="t")
                        nc.scalar.activation(
                            t_sb[:], g_ps[:], AF.Tanh, scale=inv_beta
                        )
                        tg[gi] = t_sb
                if mid_cb is not None:
                    mid_cb()
                # o-gate last: its tanh is only needed by v' (after T)
                g_ps = pp.tile([P, 2, BS], F32, tag="ps")
                for k in range(2):
                    m = 3 * 2 + k
                    nc.tensor.matmul(
                        g_ps[:, k, :], wih[:, :, ts(m, P)], x8_ap,
                        start=True, stop=not with_h, perf_mode=DR,
                    )
                    if with_h:
                        nc.tensor.matmul(
                            g_ps[:, k, :], whh[:, :, ts(m, P)],
                            v_prev[:], start=False, stop=True, perf_mode=DR,
                        )
                t_sb = gt.tile([P, 2, BS], BF16, tag="t")
                nc.scalar.activation(t_sb[:], g_ps[:], AF.Tanh, scale=inv_beta)
                tg[3] = t_sb
                return tg

            def tail(tg, s_prev, s_cur, v_cur, first):
                """Recurrent-chain ops at high priority so the scheduler's
                static per-engine orders never park bulk work (relu/prod/
                exp) in front of them.

                Cell update decomposed into TS (4x bf16 mode) + TT (2x)
                ops instead of 1x-only STT:
                  sf  = 0.5 + 0.5*tf          (TS, 4x; folds the 0.5 of s')
                  x1  = sf * s_prev           (TT on GpSimd: tf/s_prev ready
                                               early, keeps DVE free)
                  ti1 = 1 + ti                (TS, 4x)
                  x2  = ti1 * g               (TT, 2x)
                  s'  = x1 + x2               (TT, 2x)
                  v'  = (1+to)*T              (STT: fp8 out is 1x anyway)
                """
                ti, tf, g, to = tg[0], tg[1], tg[2], tg[3]
                with tc.high_priority():
                    if first:
                        # s = (1+ti)*g
                        nc.vector.scalar_tensor_tensor(
                            s_cur[:], ti[:], 1.0, g[:], OP.add, OP.mult
                        )
                    else:
                        # sf = 0.5+0.5*tf (TS, 4x — folds s'-halving); x1 =
                        # sf*s (TT, 2x); x2 via STT (chain junction: one hop
                        # after tanh-i); s' = x1+x2 (TT, 2x)
                        sf = gt2.tile([P, 2, BS], BF16, tag="sf")
                        nc.vector.tensor_scalar(
                            sf[:], tf[:], 0.5, 0.5, OP.mult, OP.add
                        )
                        x1 = gt2.tile([P, 2, BS], BF16, tag="x1")
                        nc.gpsimd.tensor_tensor(
                            x1[:], sf[:], s_prev[:], OP.mult
                        )
                        x2 = gt2.tile([P, 2, BS], BF16, tag="x2")
                        nc.vector.scalar_tensor_tensor(
                            x2[:], ti[:], 1.0, g[:], OP.add, OP.mult
                        )
                        nc.vector.tensor_tensor(s_cur[:], x1[:], x2[:], OP.add)
                    nc.scalar.activation(T_sb[:], s_cur[:], AF.Tanh, scale=0.5)
                    nc.vector.scalar_tensor_tensor(
                        v_cur[:], to[:], 1.0, T_sb[:], OP.add, OP.mult
                    )

            inv_beta = float(1.0 / _BETA)
            hid_scale = float(SH / _GAMMA)
            inv_delta = float(1.0 / DELTA)

            # ---- init: lstm(pe_0) with zero state ---------------------
            # the init cell input is pe[pos[:,0]] alone (no token embed) so
            # it gets its own x0pe stream; scan step i uses xpe slot i =
            # te[s_i] + pe_i.
            tg0 = gate_step(x0pe_sb[:], None, with_h=False, inv_beta=inv_beta)
            tail(tg0, None, s_bufs[1], v_bufs[1], first=True)

            pending = []  # deferred (step, e8, pr8) awaiting esum/pick MMs

            def flush_accum(j, e8_j, pr8_j):
                nc.tensor.matmul(
                    esum_ps[:], slide[:, :, D - 1 - j : 2 * D - 1 - j],
                    e8_j[:], start=(j == 0), stop=(j == n_steps - 1),
                    perf_mode=DR, skip_group_check=True,
                )
                nc.tensor.matmul(
                    pick_ps[:], slide[:, :, D - 1 - j : 2 * D - 1 - j],
                    pr8_j[:], start=(j == 0), stop=(j == n_steps - 1),
                    perf_mode=DR, skip_group_check=True,
                )

            # ---- scan -------------------------------------------------
            for i in range(n_steps):
                v_prev, v_cur = v_bufs[(i + 1) % 2], v_bufs[i % 2]
                s_prev, s_cur = s_bufs[(i + 1) % 2], s_bufs[i % 2]

                hid8 = []

                def mlp_front(v_prev=v_prev, hid8=hid8):
                    # W1 + relus emitted mid-gate-block: PE runs W1 right
                    # after whh-i so both relus clear DVE before the chain
                    # junction (ti1/x2/s') needs it
                    for hh in range(2):
                        h_ps = pp.tile([P, 2, BS], F32, tag="ps")
                        for k in range(2):
                            m = hh * 2 + k
                            nc.tensor.matmul(
                                h_ps[:, k, :], w1[:, :, ts(m, P)], v_prev[:],
                                start=True, stop=True, perf_mode=DR,
                            )
                        h8 = hp.tile([P, 2, BS], FP8, tag="h8")
                        # h0-relu on DVE (clears the engine before the x2/s'
                        # junction); h1-relu on ACT (fills its idle slot
                        # between tanh-i and tanh-o)
                        if hh == 0:
                            nc.vector.tensor_scalar(
                                h8[:], h_ps[:], hid_scale, 0.0, OP.mult, OP.max
                            )
                        else:
                            nc.scalar.activation(
                                h8[:], h_ps[:], AF.Relu, scale=hid_scale
                            )
                        hid8.append(h8)

                # gates + cell update FIRST (the serial chain)
                tg = gate_step(
                    xpe_sb[:, i], v_prev, with_h=True, inv_beta=inv_beta,
                    mid_cb=mlp_front,
                )
                tail(tg, s_prev, s_cur, v_cur, first=False)

                l_ps = pp.tile([P, 2, BS], F32, tag="ps")
                for t in range(2):
                    for j in range(2):
                        nc.tensor.matmul(
                            l_ps[:, t, :], w2[:, 2 * j : 2 * j + 2, ts(t, P)],
                            hid8[j][:], start=(j == 0), stop=(j == 1),
                            perf_mode=DR,
                        )
                e8 = ep.tile([P, 2, BS], FP8, tag="e8")
                nc.scalar.activation(e8[:], l_ps[:], AF.Exp, scale=inv_delta)
                # pick-product straight from the logits PSUM on DVE (one op;
                # the l_ps banks free after this + the exp read)
                pr8 = ep.tile([P, 2, BS], FP8, tag="pr8")
                nc.vector.tensor_tensor(
                    pr8[:], l_ps[:], ohs_sb[:, i], OP.mult
                )

                # esum/pick accumulation (fp8 DoubleRow; M=64 dst), deferred
                # by one step so these MMs never sit in the PE's in-order
                # queue ahead of the next step's chain-critical gate matmuls
                # while still waiting on exp/prod outputs.
                pending.append((i, e8, pr8))
                if i > 0:
                    flush_accum(*pending.pop(0))

            # ---- epilogue ---------------------------------------------
            while pending:
                flush_accum(*pending.pop(0))
            ln_e = sing.tile([D, BS], F32, tag="lne")
            nc.scalar.activation(ln_e[:], esum_ps[:], AF.Ln)
            diff = sing.tile([D, BS], F32, tag="diff")
            nc.vector.scalar_tensor_tensor(
                diff[:], pick_ps[:], inv_delta, ln_e[:],
                OP.mult, OP.subtract,
            )
            fin_ps = pp.tile([P, 2, BS], F32, tag="ps")
            nc.tensor.matmul(
                fin_ps[0:1, 0, :], ones64[:, 0:1], diff[:], start=True, stop=True
            )
            out_sb = sing.tile([1, BS], F32, tag="outsb")
            nc.scalar.activation(out_sb[:], fin_ps[0:1, 0, :], AF.Copy)
            nc.sync.dma_start(out_d, out_sb[:])

    nc.compile()
    return nc


_BETA = None
_GAMMA = None


def _compute_scales(W_ih, W_hh, W1):
    half = np.ones((4 * E, 1), np.float32)
    half[: 2 * E] = 0.5
    half[3 * E :] = 0.5
    Wg_ih = np.asarray(W_ih, np.float32) * half
    Wg_hh = np.asarray(W_hh, np.float32) * half
    beta = 216.0 / max(np.abs(Wg_ih / SX).max(), np.abs(Wg_hh / 2.0).max())
    gamma = 216.0 / np.abs(np.asarray(W1, np.float32) / 2.0).max()
    return beta, gamma, Wg_ih, Wg_hh


def prep_inputs(token_embed, W_ih, b_ih, b_hh, W_hh, W1, b1, W2, b2, pos_list,
                input_samples):
    f = np.float32
    for b in (b_ih, b_hh, b1, b2):
        assert np.all(np.asarray(b) == 0), "nonzero biases unsupported"
    beta, gamma, Wg_ih, Wg_hh = _compute_scales(W_ih, W_hh, W1)
    assert beta == _BETA and gamma == _GAMMA

    def lhsT8(Wt, ko):  # [K, M] -> [P, ko, M] fp8
        K, M = Wt.shape
        return np.ascontiguousarray(
            _q8(Wt).reshape(ko, P, M).transpose(1, 0, 2)
        )

    petab = _pe_table()
    slide = np.zeros((P, 2, 2 * D), f)
    slide[:, :, D - 1] = 1.0

    shared = {
        "wih": lhsT8(beta / SX * Wg_ih.T, 2),
        "whh": lhsT8(beta / 2.0 * Wg_hh.T, 2),
        "w1": lhsT8(gamma / 2.0 * np.asarray(W1, f).T, 2),
        "w2": lhsT8(DELTA / SH * np.asarray(W2, f).T, 4),
        "slide": _q8(slide),
        "ones64": np.ones((D, 1), f),
    }
    samples = np.asarray(input_samples)
    poss = np.asarray(pos_list)
    te_f = np.asarray(token_embed, f)  # [NCL, E]
    in_maps = []
    for c in range(NCORES):
        lo, hi = c * BS, (c + 1) * BS
        sa = samples[lo:hi]  # [BS, D]
        po = poss[lo:hi]
        ohs = np.zeros((D, 2, P, BS), NPF8)
        ii = np.arange(BS)
        for i in range(D):
            s = np.asarray(sa[:, i])
            ohs[i, s // P, s % P, ii] = 1.0
        ohs = np.ascontiguousarray(ohs.transpose(0, 2, 1, 3))
        # full LSTM input per step: x_i = te[s_i] + pe(pos_i), fp8 at 64x
        xpe = _q8(SX * (te_f[sa.T] + petab[po.T]))  # [D, BS, E]
        xpe = np.ascontiguousarray(
            xpe.transpose(0, 2, 1).reshape(D, 2, P, BS).transpose(0, 2, 1, 3)
        )
        x0pe = _q8(SX * petab[po[:, 0]])  # [BS, E] — init cell input (pe only)
        x0pe = np.ascontiguousarray(
            x0pe.T.reshape(2, P, BS).transpose(1, 0, 2)
        )
        m = dict(shared)
        m["ohs"] = ohs
        m["xpe"] = xpe
        m["x0pe"] = x0pe
        in_maps.append(m)
    return in_maps


_CACHE = {}


def kernel(**inputs) -> np.ndarray:
    global _BETA, _GAMMA
    if "nc" not in _CACHE:
        _BETA, _GAMMA, _, _ = _compute_scales(
            inputs["W_ih"], inputs["W_hh"], inputs["W1"]
        )
        _CACHE["nc"] = build_bass()
    nc = _CACHE["nc"]
    in_maps = prep_inputs(**inputs)
    res = bass_utils.run_bass_kernel_spmd(nc, in_maps, core_ids=list(range(NCORES)))
    _CACHE["last_results"] = res
    out = np.empty((B, 1), np.float32)
    for c in range(NCORES):
        out[c * BS : (c + 1) * BS, 0] = np.asarray(
            res.results[c]["out"], np.float32
        ).reshape(BS)
    return out



# revision 5
# speedup vs baseline: 1.0546x; 1.0215x over previous
"""Trainium2 Bass kernel for nn_CondRnnSampler — v3.

v2 (fp8 DoubleRow + all-tanh) was ACT-engine bound at ~9 ACTIVATE ops/step
(~1250 ns each, 95% busy); after rebalancing the kernel is chain-latency
bound (~9.9 us/step), so emission order targets the recurrence path.
v3 changes:
  - x = te[s]+pe gathered fully host-side (fp8 stream) — drops the te
    matmuls and the DVE x-adds.
  - esum/pick selector matmuls run DoubleRow over both k-halves (2 MMs/step
    instead of 4).
  - pick-product reads the logits PSUM directly on DVE (drops the ACT COPY).
  - both MLP relus on DVE, emitted via gate_step's mid_cb so their W1
    matmuls run right after whh-i on PE and the relus clear DVE before the
    chain junction (x2/s') needs it; the o-gate block comes after.
  - cell update as TS (4x) + TT (2x) + STT mix; x1 = sf*s_prev runs on
    GpSimd (off the critical path — the junction is the x2 path).
ACT per step: 4 gate tanh + state tanh + exp = 6 ops.

Per-core (512 rows), per step:
  MLP:   hid = relu(W1 h), logits = W2 hid, e = exp(logits), prod = logits*oh
  cell:  gates = W_ih x + W_hh h (fp8 DoubleRow, K=256/instr), all-tanh via
         sigma(z) = (1+tanh(z/2))/2.  State: s = 2c (bf16), v = 2h (fp8):
           s' = 0.5*(1+tf)*s + (1+ti)*g ;  v' = (1+to)*tanh(0.5 s')
  out:   esum/pick accumulate into [64,BS] PSUM banks via sliding-selector
         fp8 DoubleRow matmuls.

Scales (folded on host): x8 = 64*x, v = 2h, hid8 = 8*hid, gates PSUM = beta*a,
logits PSUM = delta*l.  One-hots (sample) and the full gathered LSTM input
(te[s]+pe) are built host-side and DMA-streamed.
"""

import sys

sys.path.insert(0, "/opt/trn_rl_repo")

from contextlib import ExitStack

import ml_dtypes
import numpy as np

import concourse.bacc as bacc
import concourse.tile as tile
from concourse import bass_utils, mybir
from concourse.bass import ts

B, D, E, NCL = 4096, 64, 256, 256
NCORES = 8
BS = B // NCORES
P = 128

AF = mybir.ActivationFunctionType
OP = mybir.AluOpType
F32 = mybir.dt.float32
BF16 = mybir.dt.bfloat16
FP8 = mybir.dt.float8e4
DR = mybir.MatmulPerfMode.DoubleRow
NPBF = ml_dtypes.bfloat16
NPF8 = ml_dtypes.float8_e4m3

SX = 64.0  # x fp8 scale
SH = 8.0  # hid fp8 scale
DELTA = 256.0  # logits PSUM scale


def _pe_table() -> np.ndarray:
    half = np.float32(E // 2)
    inv = (
        np.float32(1.0)
        / (np.float32(10000.0) ** (np.arange(E // 2, dtype=np.float32) / half))
    ).astype(np.float32)
    pos = np.arange(D, dtype=np.float32)[:, None]
    ang = pos * inv[None, :]
    return np.concatenate([np.sin(ang), np.cos(ang)], axis=1).astype(np.float32)


def _q8(x):
    return np.clip(np.asarray(x, np.float32), -240, 240).astype(NPF8)


def build_bass(n_steps: int = D):
    nc = bacc.Bacc("TRN2", debug=False, target_bir_lowering=False, num_devices=NCORES)

    def din(name, shape, dt):
        return nc.dram_tensor(name, list(shape), dt, kind="ExternalInput").ap()

    wih_d = din("wih", (P, 2, 4 * E), FP8)
    whh_d = din("whh", (P, 2, 4 * E), FP8)
    w1_d = din("w1", (P, 2, 2 * E), FP8)
    w2_d = din("w2", (P, 4, NCL), FP8)
    slide_d = din("slide", (P, 2, 2 * D), FP8)  # ones at col D-1 (both halves)
    ones64_d = din("ones64", (D, 1), F32)
    ohs_d = din("ohs", (D, P, 2, BS), FP8)  # one-hot(sample) per step
    xpe_d = din("xpe", (D, P, 2, BS), FP8)  # 64*(te[s]+petab[pos]) per step
    x0pe_d = din("x0pe", (P, 2, BS), FP8)  # 64*petab[pos_0] (init cell input)
    out_d = nc.dram_tensor("out", [1, BS], F32, kind="ExternalOutput").ap()

    with tile.TileContext(nc) as tc:
        with ExitStack() as ctx:
            sing = ctx.enter_context(tc.tile_pool(name="sing", bufs=1))
            gt = ctx.enter_context(tc.tile_pool(name="gt", bufs=6))
            gt2 = ctx.enter_context(tc.tile_pool(name="gt2", bufs=2))
            hp = ctx.enter_context(tc.tile_pool(name="hp", bufs=3))
            ep = ctx.enter_context(tc.tile_pool(name="ep", bufs=4))
            psing = ctx.enter_context(tc.tile_pool(name="psing", bufs=1, space="PSUM"))
            pp = ctx.enter_context(tc.tile_pool(name="pp", bufs=3, space="PSUM"))

            # ---- resident tensors -------------------------------------
            # init-critical first: step-0 one-hot/pe slices + gate weights
            ohs_sb = sing.tile([P, D, 2, BS], FP8, tag="ohs")
            xpe_sb = sing.tile([P, D, 2, BS], FP8, tag="xpe")
            x0pe_sb = sing.tile([P, 2, BS], FP8, tag="x0pe")
            nc.sync.dma_start(x0pe_sb[:], x0pe_d)
            nc.sync.dma_start(xpe_sb[:, 0], xpe_d[0])
            nc.sync.dma_start(ohs_sb[:, 0], ohs_d[0])
            wih = sing.tile([P, 2, 4 * E], FP8, tag="wih")
            nc.sync.dma_start(wih[:], wih_d)
            whh = sing.tile([P, 2, 4 * E], FP8, tag="whh")
            nc.sync.dma_start(whh[:], whh_d)
            w1 = sing.tile([P, 2, 2 * E], FP8, tag="w1")
            nc.sync.dma_start(w1[:], w1_d)
            w2 = sing.tile([P, 4, NCL], FP8, tag="w2")
            nc.sync.dma_start(w2[:], w2_d)
            slide = sing.tile([P, 2, 2 * D], FP8, tag="slide")
            nc.sync.dma_start(slide[:], slide_d)
            ones64 = sing.tile([D, 1], F32, tag="ones64")
            nc.sync.dma_start(ones64[:], ones64_d)

            for i in range(1, n_steps):
                nc.sync.dma_start(ohs_sb[:, i], ohs_d[i])
                nc.sync.dma_start(xpe_sb[:, i], xpe_d[i])

            # double-buffered recurrent state (parity by step)
            s_bufs = [
                sing.tile([P, 2, BS], BF16, tag=f"s{j}", name=f"s{j}")
                for j in range(2)
            ]
            v_bufs = [
                sing.tile([P, 2, BS], FP8, tag=f"v{j}", name=f"v{j}")
                for j in range(2)
            ]
            T_sb = sing.tile([P, 2, BS], BF16, tag="T")
            esum_ps = psing.tile([D, BS], F32, tag="esum")
            pick_ps = psing.tile([D, BS], F32, tag="pick")

            # scales arrive via sc tile? No - bake as python floats at build:
            # (they depend only on weight maxima; recomputed per call would
            # need rebuild. Instead scales are fixed: beta/gamma baked by
            # prep_inputs to match BETA/GAMMA globals.)

            def gate_step(x8_ap, v_prev, with_h, inv_beta, mid_cb=None):
                """gates -> t tiles [ti, tf, g, to]; order f,g,i,o so the
                chain ops X1 (needs tf) and X2 (needs g) unblock earliest.

                mid_cb (default-priority emissions) runs between the i- and
                o-gate blocks: the W1 matmuls land on PE right after whh-i,
                so the relus become ready (and clear DVE) well before the
                ti1/x2/s' chain junction needs the engine."""
                tg = [None] * 4
                # v-independent wih matmuls for the chain-leading f/g gates
                # are emitted at normal priority AHEAD of any whh matmul, so
                # the in-order PE queue runs them during the v-wait bubble
                # instead of stalling behind the first v-dependent whh.
                pre = {}
                if with_h:
                    for gi in (1, 2, 0):  # f, g, i
                        g_ps = pp.tile([P, 2, BS], F32, tag="ps")
                        for k in range(2):
                            nc.tensor.matmul(
                                g_ps[:, k, :], wih[:, :, ts(gi * 2 + k, P)],
                                x8_ap, start=True, stop=False, perf_mode=DR,
                            )
                        pre[gi] = g_ps
                with tc.high_priority():
                    for gi in (1, 2, 0):  # f, g, i
                        if gi in pre:
                            g_ps = pre[gi]
                            for k in range(2):
                                nc.tensor.matmul(
                                    g_ps[:, k, :], whh[:, :, ts(gi * 2 + k, P)],
                                    v_prev[:], start=False, stop=True,
                                    perf_mode=DR,
                                )
                        else:
                            g_ps = pp.tile([P, 2, BS], F32, tag="ps")
                            for k in range(2):
                                m = gi * 2 + k
                                nc.tensor.matmul(
                                    g_ps[:, k, :], wih[:, :, ts(m, P)], x8_ap,
                                    start=True, stop=not with_h, perf_mode=DR,
                                )
                                if with_h:
                                    nc.tensor.matmul(
                                        g_ps[:, k, :], whh[:, :, ts(m, P)],
                                        v_prev[:], start=False, stop=True,
                                        perf_mode=DR,
                                    )
                        t_sb = gt.tile([P, 2, BS], BF16, tag="t")
                        nc.scalar.activation(
                            t_sb[:], g_ps[:], AF.Tanh, scale=inv_beta
                        )
                        tg[gi] = t_sb
                if mid_cb is not None:
                    mid_cb()
                # o-gate last: its tanh is only needed by v' (after T)
                g_ps = pp.tile([P, 2, BS], F32, tag="ps")
                for k in range(2):
                    m = 3 * 2 + k
                    nc.tensor.matmul(
                        g_ps[:, k, :], wih[:, :, ts(m, P)], x8_ap,
                        start=True, stop=not with_h, perf_mode=DR,
                    )
                    if with_h:
                        nc.tensor.matmul(
                            g_ps[:, k, :], whh[:, :, ts(m, P)],
                            v_prev[:], start=False, stop=True, perf_mode=DR,
                        )
                t_sb = gt.tile([P, 2, BS], BF16, tag="t")
                nc.scalar.activation(t_sb[:], g_ps[:], AF.Tanh, scale=inv_beta)
                tg[3] = t_sb
                return tg

            def tail(tg, s_prev, s_cur, v_cur, first):
                """Recurrent-chain ops at high priority so the scheduler's
                static per-engine orders never park bulk work (relu/prod/
                exp) in front of them.

                Cell update decomposed into TS (4x bf16 mode) + TT (2x)
                ops instead of 1x-only STT:
                  sf  = 0.5 + 0.5*tf          (TS, 4x; folds the 0.5 of s')
                  x1  = sf * s_prev           (TT on GpSimd: tf/s_prev ready
                                               early, keeps DVE free)
                  ti1 = 1 + ti                (TS, 4x)
                  x2  = ti1 * g               (TT, 2x)
                  s'  = x1 + x2               (TT, 2x)
                  v'  = (1+to)*T              (STT: fp8 out is 1x anyway)
                """
                ti, tf, g, to = tg[0], tg[1], tg[2], tg[3]
                with tc.high_priority():
                    if first:
                        # s = (1+ti)*g
                        nc.vector.scalar_tensor_tensor(
                            s_cur[:], ti[:], 1.0, g[:], OP.add, OP.mult
                        )
                    else:
                        # sf = 0.5+0.5*tf (TS, 4x — folds s'-halving); x1 =
                        # sf*s (TT, 2x); x2 via STT (chain junction: one hop
                        # after tanh-i); s' = x1+x2 (TT, 2x)
                        sf = gt2.tile([P, 2, BS], BF16, tag="sf")
                        nc.vector.tensor_scalar(
                            sf[:], tf[:], 0.5, 0.5, OP.mult, OP.add
                        )
                        x1 = gt2.tile([P, 2, BS], BF16, tag="x1")
                        nc.gpsimd.tensor_tensor(
                            x1[:], sf[:], s_prev[:], OP.mult
                        )
                        x2 = gt2.tile([P, 2, BS], BF16, tag="x2")
                        nc.vector.scalar_tensor_tensor(
                            x2[:], ti[:], 1.0, g[:], OP.add, OP.mult
                        )
                        nc.vector.tensor_tensor(s_cur[:], x1[:], x2[:], OP.add)
                    nc.scalar.activation(T_sb[:], s_cur[:], AF.Tanh, scale=0.5)
                    nc.vector.scalar_tensor_tensor(
                        v_cur[:], to[:], 1.0, T_sb[:], OP.add, OP.mult
                    )

            inv_beta = float(1.0 / _BETA)
            hid_scale = float(SH / _GAMMA)
            inv_delta = float(1.0 / DELTA)

            # ---- init: lstm(pe_0) with zero state ---------------------
            # the init cell input is pe[pos[:,0]] alone (no token embed) so
            # it gets its own x0pe stream; scan step i uses xpe slot i =
            # te[s_i] + pe_i.
            tg0 = gate_step(x0pe_sb[:], None, with_h=False, inv_beta=inv_beta)
            tail(tg0, None, s_bufs[1], v_bufs[1], first=True)

            pending = []  # deferred (step, e8, pr8) awaiting esum/pick MMs

            def flush_accum(j, e8_j, pr8_j):
                nc.tensor.matmul(
                    esum_ps[:], slide[:, :, D - 1 - j : 2 * D - 1 - j],
                    e8_j[:], start=(j == 0), stop=(j == n_steps - 1),
                    perf_mode=DR, skip_group_check=True,
                )
                nc.tensor.matmul(
                    pick_ps[:], slide[:, :, D - 1 - j : 2 * D - 1 - j],
                    pr8_j[:], start=(j == 0), stop=(j == n_steps - 1),
                    perf_mode=DR, skip_group_check=True,
                )

            # ---- scan -------------------------------------------------
            for i in range(n_steps):
                v_prev, v_cur = v_bufs[(i + 1) % 2], v_bufs[i % 2]
                s_prev, s_cur = s_bufs[(i + 1) % 2], s_bufs[i % 2]

                hid8 = []
                h1_ps = []

                def mlp_front(v_prev=v_prev, hid8=hid8, h1_ps=h1_ps):
                    # W1 matmuls emitted mid-gate-block: PE runs them right
                    # after whh-i.  h0-relu on DVE here (clears the engine
                    # before the x2/s' junction); h1-relu is emitted after
                    # the o-gate so the ACT order is f,g,i,o,relu1,T.
                    for hh in range(2):
                        h_ps = pp.tile([P, 2, BS], F32, tag="ps")
                        for k in range(2):
                            m = hh * 2 + k
                            nc.tensor.matmul(
                                h_ps[:, k, :], w1[:, :, ts(m, P)], v_prev[:],
                                start=True, stop=True, perf_mode=DR,
                            )
                        if hh == 0:
                            h8 = hp.tile([P, 2, BS], FP8, tag="h8")
                            nc.vector.tensor_scalar(
                                h8[:], h_ps[:], hid_scale, 0.0, OP.mult, OP.max
                            )
                            hid8.append(h8)
                        else:
                            h1_ps.append(h_ps)

                # gates + cell update FIRST (the serial chain)
                tg = gate_step(
                    xpe_sb[:, i], v_prev, with_h=True, inv_beta=inv_beta,
                    mid_cb=mlp_front,
                )
                h8b = hp.tile([P, 2, BS], FP8, tag="h8")
                nc.scalar.activation(
                    h8b[:], h1_ps[0][:], AF.Relu, scale=hid_scale
                )
                hid8.append(h8b)
                tail(tg, s_prev, s_cur, v_cur, first=False)

                l_ps = pp.tile([P, 2, BS], F32, tag="ps")
                for t in range(2):
                    for j in range(2):
                        nc.tensor.matmul(
                            l_ps[:, t, :], w2[:, 2 * j : 2 * j + 2, ts(t, P)],
                            hid8[j][:], start=(j == 0), stop=(j == 1),
                            perf_mode=DR,
                        )
                e8 = ep.tile([P, 2, BS], FP8, tag="e8")
                nc.scalar.activation(e8[:], l_ps[:], AF.Exp, scale=inv_delta)
                # pick-product straight from the logits PSUM on DVE (one op;
                # the l_ps banks free after this + the exp read)
                pr8 = ep.tile([P, 2, BS], FP8, tag="pr8")
                nc.vector.tensor_tensor(
                    pr8[:], l_ps[:], ohs_sb[:, i], OP.mult
                )

                # esum/pick accumulation (fp8 DoubleRow; M=64 dst), deferred
                # by one step so these MMs never sit in the PE's in-order
                # queue ahead of the next step's chain-critical gate matmuls
                # while still waiting on exp/prod outputs.
                pending.append((i, e8, pr8))
                if i > 0:
                    flush_accum(*pending.pop(0))

            # ---- epilogue ---------------------------------------------
            while pending:
                flush_accum(*pending.pop(0))
            ln_e = sing.tile([D, BS], F32, tag="lne")
            nc.scalar.activation(ln_e[:], esum_ps[:], AF.Ln)
            diff = sing.tile([D, BS], F32, tag="diff")
            nc.vector.scalar_tensor_tensor(
                diff[:], pick_ps[:], inv_delta, ln_e[:],
                OP.mult, OP.subtract,
            )
            fin_ps = pp.tile([P, 2, BS], F32, tag="ps")
            nc.tensor.matmul(
                fin_ps[0:1, 0, :], ones64[:, 0:1], diff[:], start=True, stop=True
            )
            out_sb = sing.tile([1, BS], F32, tag="outsb")
            nc.scalar.activation(out_sb[:], fin_ps[0:1, 0, :], AF.Copy)
            nc.sync.dma_start(out_d, out_sb[:])

    nc.compile()
    return nc


_BETA = None
_GAMMA = None


def _compute_scales(W_ih, W_hh, W1):
    half = np.ones((4 * E, 1), np.float32)
    half[: 2 * E] = 0.5
    half[3 * E :] = 0.5
    Wg_ih = np.asarray(W_ih, np.float32) * half
    Wg_hh = np.asarray(W_hh, np.float32) * half
    beta = 216.0 / max(np.abs(Wg_ih / SX).max(), np.abs(Wg_hh / 2.0).max())
    gamma = 216.0 / np.abs(np.asarray(W1, np.float32) / 2.0).max()
    return beta, gamma, Wg_ih, Wg_hh


def prep_inputs(token_embed, W_ih, b_ih, b_hh, W_hh, W1, b1, W2, b2, pos_list,
                input_samples):
    f = np.float32
    for b in (b_ih, b_hh, b1, b2):
        assert np.all(np.asarray(b) == 0), "nonzero biases unsupported"
    beta, gamma, Wg_ih, Wg_hh = _compute_scales(W_ih, W_hh, W1)
    assert beta == _BETA and gamma == _GAMMA

    def lhsT8(Wt, ko):  # [K, M] -> [P, ko, M] fp8
        K, M = Wt.shape
        return np.ascontiguousarray(
            _q8(Wt).reshape(ko, P, M).transpose(1, 0, 2)
        )

    petab = _pe_table()
    slide = np.zeros((P, 2, 2 * D), f)
    slide[:, :, D - 1] = 1.0

    shared = {
        "wih": lhsT8(beta / SX * Wg_ih.T, 2),
        "whh": lhsT8(beta / 2.0 * Wg_hh.T, 2),
        "w1": lhsT8(gamma / 2.0 * np.asarray(W1, f).T, 2),
        "w2": lhsT8(DELTA / SH * np.asarray(W2, f).T, 4),
        "slide": _q8(slide),
        "ones64": np.ones((D, 1), f),
    }
    samples = np.asarray(input_samples)
    poss = np.asarray(pos_list)
    te_f = np.asarray(token_embed, f)  # [NCL, E]
    in_maps = []
    for c in range(NCORES):
        lo, hi = c * BS, (c + 1) * BS
        sa = samples[lo:hi]  # [BS, D]
        po = poss[lo:hi]
        ohs = np.zeros((D, 2, P, BS), NPF8)
        ii = np.arange(BS)
        for i in range(D):
            s = np.asarray(sa[:, i])
            ohs[i, s // P, s % P, ii] = 1.0
        ohs = np.ascontiguousarray(ohs.transpose(0, 2, 1, 3))
        # full LSTM input per step: x_i = te[s_i] + pe(pos_i), fp8 at 64x
        xpe = _q8(SX * (te_f[sa.T] + petab[po.T]))  # [D, BS, E]
        xpe = np.ascontiguousarray(
            xpe.transpose(0, 2, 1).reshape(D, 2, P, BS).transpose(0, 2, 1, 3)
        )
        x0pe = _q8(SX * petab[po[:, 0]])  # [BS, E] — init cell input (pe only)
        x0pe = np.ascontiguousarray(
            x0pe.T.reshape(2, P, BS).transpose(1, 0, 2)
        )
        m = dict(shared)
        m["ohs"] = ohs
        m["xpe"] = xpe
        m["x0pe"] = x0pe
        in_maps.append(m)
    return in_maps


_CACHE = {}


def kernel(**inputs) -> np.ndarray:
    global _BETA, _GAMMA
    if "nc" not in _CACHE:
        _BETA, _GAMMA, _, _ = _compute_scales(
            inputs["W_ih"], inputs["W_hh"], inputs["W1"]
        )
        _CACHE["nc"] = build_bass()
    nc = _CACHE["nc"]
    in_maps = prep_inputs(**inputs)
    res = bass_utils.run_bass_kernel_spmd(nc, in_maps, core_ids=list(range(NCORES)))
    _CACHE["last_results"] = res
    out = np.empty((B, 1), np.float32)
    for c in range(NCORES):
        out[c * BS : (c + 1) * BS, 0] = np.asarray(
            res.results[c]["out"], np.float32
        ).reshape(BS)
    return out



# revision 6
# speedup vs baseline: 1.0904x; 1.0339x over previous
"""Trainium2 Bass kernel for nn_CondRnnSampler — v3.

v2 (fp8 DoubleRow + all-tanh) was ACT-engine bound at ~9 ACTIVATE ops/step
(~1250 ns each, 95% busy); after rebalancing the kernel is chain-latency
bound (~9.9 us/step), so emission order targets the recurrence path.
v3 changes:
  - x = te[s]+pe gathered fully host-side (fp8 stream) — drops the te
    matmuls and the DVE x-adds.
  - esum/pick selector matmuls run DoubleRow over both k-halves (2 MMs/step
    instead of 4).
  - pick-product reads the logits PSUM directly on DVE (drops the ACT COPY).
  - both MLP relus on DVE, emitted via gate_step's mid_cb so their W1
    matmuls run right after whh-i on PE and the relus clear DVE before the
    chain junction (x2/s') needs it; the o-gate block comes after.
  - cell update as TS (4x) + TT (2x) + STT mix; x1 = sf*s_prev runs on
    GpSimd (off the critical path — the junction is the x2 path).
ACT per step: 4 gate tanh + state tanh + exp = 6 ops.

Per-core (512 rows), per step:
  MLP:   hid = relu(W1 h), logits = W2 hid, e = exp(logits), prod = logits*oh
  cell:  gates = W_ih x + W_hh h (fp8 DoubleRow, K=256/instr), all-tanh via
         sigma(z) = (1+tanh(z/2))/2.  State: s = 2c (bf16), v = 2h (fp8):
           s' = 0.5*(1+tf)*s + (1+ti)*g ;  v' = (1+to)*tanh(0.5 s')
  out:   esum/pick accumulate into [64,BS] PSUM banks via sliding-selector
         fp8 DoubleRow matmuls.

Scales (folded on host): x8 = 64*x, v = 2h, hid8 = 8*hid, gates PSUM = beta*a,
logits PSUM = delta*l.  One-hots (sample) and the full gathered LSTM input
(te[s]+pe) are built host-side and DMA-streamed.
"""

import sys

sys.path.insert(0, "/opt/trn_rl_repo")

from contextlib import ExitStack

import ml_dtypes
import numpy as np

import concourse.bacc as bacc
import concourse.tile as tile
from concourse import bass_utils, mybir
from concourse.bass import ts

B, D, E, NCL = 4096, 64, 256, 256
NCORES = 8
BS = B // NCORES
P = 128

AF = mybir.ActivationFunctionType
OP = mybir.AluOpType
F32 = mybir.dt.float32
BF16 = mybir.dt.bfloat16
FP8 = mybir.dt.float8e4
DR = mybir.MatmulPerfMode.DoubleRow
NPBF = ml_dtypes.bfloat16
NPF8 = ml_dtypes.float8_e4m3

SX = 64.0  # x fp8 scale
SH = 8.0  # hid fp8 scale
DELTA = 256.0  # logits PSUM scale


def _pe_table() -> np.ndarray:
    half = np.float32(E // 2)
    inv = (
        np.float32(1.0)
        / (np.float32(10000.0) ** (np.arange(E // 2, dtype=np.float32) / half))
    ).astype(np.float32)
    pos = np.arange(D, dtype=np.float32)[:, None]
    ang = pos * inv[None, :]
    return np.concatenate([np.sin(ang), np.cos(ang)], axis=1).astype(np.float32)


def _q8(x):
    return np.clip(np.asarray(x, np.float32), -240, 240).astype(NPF8)


def build_bass(n_steps: int = D):
    nc = bacc.Bacc("TRN2", debug=False, target_bir_lowering=False, num_devices=NCORES)

    def din(name, shape, dt):
        return nc.dram_tensor(name, list(shape), dt, kind="ExternalInput").ap()

    wih_d = din("wih", (P, 2, 4 * E), FP8)
    whh_d = din("whh", (P, 2, 4 * E), FP8)
    w1_d = din("w1", (P, 2, 2 * E), FP8)
    w2_d = din("w2", (P, 4, NCL), FP8)
    slide_d = din("slide", (P, 2, 2 * D), FP8)  # ones at col D-1 (both halves)
    ones64_d = din("ones64", (D, 1), F32)
    ohs_d = din("ohs", (D, P, 2, BS), FP8)  # one-hot(sample) per step
    xpe_d = din("xpe", (D, P, 2, BS), FP8)  # 64*(te[s]+petab[pos]) per step
    x0pe_d = din("x0pe", (P, 2, BS), FP8)  # 64*petab[pos_0] (init cell input)
    out_d = nc.dram_tensor("out", [1, BS], F32, kind="ExternalOutput").ap()

    with tile.TileContext(nc) as tc:
        with ExitStack() as ctx:
            sing = ctx.enter_context(tc.tile_pool(name="sing", bufs=1))
            gt = ctx.enter_context(tc.tile_pool(name="gt", bufs=6))
            gt2 = ctx.enter_context(tc.tile_pool(name="gt2", bufs=2))
            hp = ctx.enter_context(tc.tile_pool(name="hp", bufs=3))
            ep = ctx.enter_context(tc.tile_pool(name="ep", bufs=4))
            psing = ctx.enter_context(tc.tile_pool(name="psing", bufs=1, space="PSUM"))
            pp = ctx.enter_context(tc.tile_pool(name="pp", bufs=3, space="PSUM"))

            # ---- resident tensors -------------------------------------
            # init-critical first: step-0 one-hot/pe slices + gate weights
            ohs_sb = sing.tile([P, D, 2, BS], FP8, tag="ohs")
            xpe_sb = sing.tile([P, D, 2, BS], FP8, tag="xpe")
            x0pe_sb = sing.tile([P, 2, BS], FP8, tag="x0pe")
            nc.sync.dma_start(x0pe_sb[:], x0pe_d)
            nc.sync.dma_start(xpe_sb[:, 0], xpe_d[0])
            nc.sync.dma_start(ohs_sb[:, 0], ohs_d[0])
            wih = sing.tile([P, 2, 4 * E], FP8, tag="wih")
            nc.sync.dma_start(wih[:], wih_d)
            whh = sing.tile([P, 2, 4 * E], FP8, tag="whh")
            nc.sync.dma_start(whh[:], whh_d)
            w1 = sing.tile([P, 2, 2 * E], FP8, tag="w1")
            nc.sync.dma_start(w1[:], w1_d)
            w2 = sing.tile([P, 4, NCL], FP8, tag="w2")
            nc.sync.dma_start(w2[:], w2_d)
            slide = sing.tile([P, 2, 2 * D], FP8, tag="slide")
            nc.sync.dma_start(slide[:], slide_d)
            ones64 = sing.tile([D, 1], F32, tag="ones64")
            nc.sync.dma_start(ones64[:], ones64_d)

            for i in range(1, n_steps):
                nc.sync.dma_start(ohs_sb[:, i], ohs_d[i])
                nc.sync.dma_start(xpe_sb[:, i], xpe_d[i])

            # double-buffered recurrent state (parity by step)
            s_bufs = [
                sing.tile([P, 2, BS], BF16, tag=f"s{j}", name=f"s{j}")
                for j in range(2)
            ]
            v_bufs = [
                sing.tile([P, 2, BS], FP8, tag=f"v{j}", name=f"v{j}")
                for j in range(2)
            ]
            T_sb = sing.tile([P, 2, BS], BF16, tag="T")
            esum_ps = psing.tile([D, BS], F32, tag="esum")
            pick_ps = psing.tile([D, BS], F32, tag="pick")

            # scales arrive via sc tile? No - bake as python floats at build:
            # (they depend only on weight maxima; recomputed per call would
            # need rebuild. Instead scales are fixed: beta/gamma baked by
            # prep_inputs to match BETA/GAMMA globals.)

            def gate_step(x8_ap, v_prev, with_h, inv_beta, mid_cb=None):
                """gates -> t tiles [ti, tf, g, to]; order f,g,i,o so the
                chain ops X1 (needs tf) and X2 (needs g) unblock earliest.

                mid_cb (default-priority emissions) runs between the i- and
                o-gate blocks: the W1 matmuls land on PE right after whh-i,
                so the relus become ready (and clear DVE) well before the
                ti1/x2/s' chain junction needs the engine."""
                tg = [None] * 4
                # v-independent wih matmuls for the chain-leading f/g gates
                # are emitted at normal priority AHEAD of any whh matmul, so
                # the in-order PE queue runs them during the v-wait bubble
                # instead of stalling behind the first v-dependent whh.
                pre = {}
                if with_h:
                    for gi in (1, 2, 0):  # f, g, i
                        g_ps = pp.tile([P, 2, BS], F32, tag="ps")
                        for k in range(2):
                            nc.tensor.matmul(
                                g_ps[:, k, :], wih[:, :, ts(gi * 2 + k, P)],
                                x8_ap, start=True, stop=False, perf_mode=DR,
                            )
                        pre[gi] = g_ps
                with tc.high_priority():
                    for gi in (1, 2, 0):  # f, g, i
                        if gi in pre:
                            g_ps = pre[gi]
                            for k in range(2):
                                nc.tensor.matmul(
                                    g_ps[:, k, :], whh[:, :, ts(gi * 2 + k, P)],
                                    v_prev[:], start=False, stop=True,
                                    perf_mode=DR,
                                )
                        else:
                            g_ps = pp.tile([P, 2, BS], F32, tag="ps")
                            for k in range(2):
                                m = gi * 2 + k
                                nc.tensor.matmul(
                                    g_ps[:, k, :], wih[:, :, ts(m, P)], x8_ap,
                                    start=True, stop=not with_h, perf_mode=DR,
                                )
                                if with_h:
                                    nc.tensor.matmul(
                                        g_ps[:, k, :], whh[:, :, ts(m, P)],
                                        v_prev[:], start=False, stop=True,
                                        perf_mode=DR,
                                    )
                        t_sb = gt.tile([P, 2, BS], BF16, tag="t")
                        nc.scalar.activation(
                            t_sb[:], g_ps[:], AF.Tanh, scale=inv_beta
                        )
                        tg[gi] = t_sb
                if mid_cb is not None:
                    mid_cb()
                # o-gate last: its tanh is only needed by v' (after T)
                g_ps = pp.tile([P, 2, BS], F32, tag="ps")
                for k in range(2):
                    m = 3 * 2 + k
                    nc.tensor.matmul(
                        g_ps[:, k, :], wih[:, :, ts(m, P)], x8_ap,
                        start=True, stop=not with_h, perf_mode=DR,
                    )
                    if with_h:
                        nc.tensor.matmul(
                            g_ps[:, k, :], whh[:, :, ts(m, P)],
                            v_prev[:], start=False, stop=True, perf_mode=DR,
                        )
                t_sb = gt.tile([P, 2, BS], BF16, tag="t")
                nc.scalar.activation(t_sb[:], g_ps[:], AF.Tanh, scale=inv_beta)
                tg[3] = t_sb
                return tg

            def tail(tg, s_prev, s_cur, v_cur, first):
                """Recurrent-chain ops at high priority so the scheduler's
                static per-engine orders never park bulk work (relu/prod/
                exp) in front of them.

                Cell update decomposed into TS (4x bf16 mode) + TT (2x)
                ops instead of 1x-only STT:
                  sf  = 0.5 + 0.5*tf          (TS, 4x; folds the 0.5 of s')
                  x1  = sf * s_prev           (TT on GpSimd: tf/s_prev ready
                                               early, keeps DVE free)
                  ti1 = 1 + ti                (TS, 4x)
                  x2  = ti1 * g               (TT, 2x)
                  s'  = x1 + x2               (TT, 2x)
                  v'  = (1+to)*T              (STT: fp8 out is 1x anyway)
                """
                ti, tf, g, to = tg[0], tg[1], tg[2], tg[3]
                with tc.high_priority():
                    if first:
                        # s = (1+ti)*g
                        nc.vector.scalar_tensor_tensor(
                            s_cur[:], ti[:], 1.0, g[:], OP.add, OP.mult
                        )
                    else:
                        # sf = 0.5+0.5*tf (TS, 4x — folds s'-halving); x1 =
                        # sf*s (TT, 2x); x2 via STT (chain junction: one hop
                        # after tanh-i); s' = x1+x2 (TT, 2x)
                        sf = gt2.tile([P, 2, BS], BF16, tag="sf")
                        nc.vector.tensor_scalar(
                            sf[:], tf[:], 0.5, 0.5, OP.mult, OP.add
                        )
                        x1 = gt2.tile([P, 2, BS], BF16, tag="x1")
                        nc.gpsimd.tensor_tensor(
                            x1[:], sf[:], s_prev[:], OP.mult
                        )
                        ti1 = gt2.tile([P, 2, BS], BF16, tag="ti1")
                        nc.vector.tensor_scalar(
                            ti1[:], ti[:], 1.0, None, OP.add
                        )
                        x2 = gt2.tile([P, 2, BS], BF16, tag="x2")
                        nc.vector.tensor_tensor(x2[:], ti1[:], g[:], OP.mult)
                        nc.vector.tensor_tensor(s_cur[:], x1[:], x2[:], OP.add)
                    nc.scalar.activation(T_sb[:], s_cur[:], AF.Tanh, scale=0.5)
                    nc.vector.scalar_tensor_tensor(
                        v_cur[:], to[:], 1.0, T_sb[:], OP.add, OP.mult
                    )

            inv_beta = float(1.0 / _BETA)
            hid_scale = float(SH / _GAMMA)
            inv_delta = float(1.0 / DELTA)

            # ---- init: lstm(pe_0) with zero state ---------------------
            # the init cell input is pe[pos[:,0]] alone (no token embed) so
            # it gets its own x0pe stream; scan step i uses xpe slot i =
            # te[s_i] + pe_i.
            tg0 = gate_step(x0pe_sb[:], None, with_h=False, inv_beta=inv_beta)
            tail(tg0, None, s_bufs[1], v_bufs[1], first=True)

            pending = []  # deferred (step, e8, pr8) awaiting esum/pick MMs

            def flush_accum(j, e8_j, pr8_j):
                nc.tensor.matmul(
                    esum_ps[:], slide[:, :, D - 1 - j : 2 * D - 1 - j],
                    e8_j[:], start=(j == 0), stop=(j == n_steps - 1),
                    perf_mode=DR, skip_group_check=True,
                )
                nc.tensor.matmul(
                    pick_ps[:], slide[:, :, D - 1 - j : 2 * D - 1 - j],
                    pr8_j[:], start=(j == 0), stop=(j == n_steps - 1),
                    perf_mode=DR, skip_group_check=True,
                )

            # ---- scan -------------------------------------------------
            for i in range(n_steps):
                v_prev, v_cur = v_bufs[(i + 1) % 2], v_bufs[i % 2]
                s_prev, s_cur = s_bufs[(i + 1) % 2], s_bufs[i % 2]

                hid8 = []
                h1_ps = []

                def mlp_front(v_prev=v_prev, hid8=hid8, h1_ps=h1_ps):
                    # W1 matmuls emitted mid-gate-block: PE runs them right
                    # after whh-i.  h0-relu on DVE here (clears the engine
                    # before the x2/s' junction); h1-relu is emitted after
                    # the o-gate so the ACT order is f,g,i,o,relu1,T.
                    for hh in range(2):
                        h_ps = pp.tile([P, 2, BS], F32, tag="ps")
                        for k in range(2):
                            m = hh * 2 + k
                            nc.tensor.matmul(
                                h_ps[:, k, :], w1[:, :, ts(m, P)], v_prev[:],
                                start=True, stop=True, perf_mode=DR,
                            )
                        if hh == 0:
                            h8 = hp.tile([P, 2, BS], FP8, tag="h8")
                            nc.vector.tensor_scalar(
                                h8[:], h_ps[:], hid_scale, 0.0, OP.mult, OP.max
                            )
                            hid8.append(h8)
                        else:
                            h1_ps.append(h_ps)

                # gates + cell update FIRST (the serial chain)
                tg = gate_step(
                    xpe_sb[:, i], v_prev, with_h=True, inv_beta=inv_beta,
                    mid_cb=mlp_front,
                )
                h8b = hp.tile([P, 2, BS], FP8, tag="h8")
                nc.scalar.activation(
                    h8b[:], h1_ps[0][:], AF.Relu, scale=hid_scale
                )
                hid8.append(h8b)
                tail(tg, s_prev, s_cur, v_cur, first=False)

                l_ps = pp.tile([P, 2, BS], F32, tag="ps")
                for t in range(2):
                    for j in range(2):
                        nc.tensor.matmul(
                            l_ps[:, t, :], w2[:, 2 * j : 2 * j + 2, ts(t, P)],
                            hid8[j][:], start=(j == 0), stop=(j == 1),
                            perf_mode=DR,
                        )
                e8 = ep.tile([P, 2, BS], FP8, tag="e8")
                nc.scalar.activation(e8[:], l_ps[:], AF.Exp, scale=inv_delta)
                # pick-product straight from the logits PSUM on DVE (one op;
                # the l_ps banks free after this + the exp read)
                pr8 = ep.tile([P, 2, BS], FP8, tag="pr8")
                nc.vector.tensor_tensor(
                    pr8[:], l_ps[:], ohs_sb[:, i], OP.mult
                )

                # esum/pick accumulation (fp8 DoubleRow; M=64 dst), deferred
                # by one step so these MMs never sit in the PE's in-order
                # queue ahead of the next step's chain-critical gate matmuls
                # while still waiting on exp/prod outputs.
                pending.append((i, e8, pr8))
                if i > 0:
                    flush_accum(*pending.pop(0))

            # ---- epilogue ---------------------------------------------
            while pending:
                flush_accum(*pending.pop(0))
            ln_e = sing.tile([D, BS], F32, tag="lne")
            nc.scalar.activation(ln_e[:], esum_ps[:], AF.Ln)
            diff = sing.tile([D, BS], F32, tag="diff")
            nc.vector.scalar_tensor_tensor(
                diff[:], pick_ps[:], inv_delta, ln_e[:],
                OP.mult, OP.subtract,
            )
            fin_ps = pp.tile([P, 2, BS], F32, tag="ps")
            nc.tensor.matmul(
                fin_ps[0:1, 0, :], ones64[:, 0:1], diff[:], start=True, stop=True
            )
            out_sb = sing.tile([1, BS], F32, tag="outsb")
            nc.scalar.activation(out_sb[:], fin_ps[0:1, 0, :], AF.Copy)
            nc.sync.dma_start(out_d, out_sb[:])

    nc.compile()
    return nc


_BETA = None
_GAMMA = None


def _compute_scales(W_ih, W_hh, W1):
    half = np.ones((4 * E, 1), np.float32)
    half[: 2 * E] = 0.5
    half[3 * E :] = 0.5
    Wg_ih = np.asarray(W_ih, np.float32) * half
    Wg_hh = np.asarray(W_hh, np.float32) * half
    beta = 216.0 / max(np.abs(Wg_ih / SX).max(), np.abs(Wg_hh / 2.0).max())
    gamma = 216.0 / np.abs(np.asarray(W1, np.float32) / 2.0).max()
    return beta, gamma, Wg_ih, Wg_hh


def prep_inputs(token_embed, W_ih, b_ih, b_hh, W_hh, W1, b1, W2, b2, pos_list,
                input_samples):
    f = np.float32
    for b in (b_ih, b_hh, b1, b2):
        assert np.all(np.asarray(b) == 0), "nonzero biases unsupported"
    beta, gamma, Wg_ih, Wg_hh = _compute_scales(W_ih, W_hh, W1)
    assert beta == _BETA and gamma == _GAMMA

    def lhsT8(Wt, ko):  # [K, M] -> [P, ko, M] fp8
        K, M = Wt.shape
        return np.ascontiguousarray(
            _q8(Wt).reshape(ko, P, M).transpose(1, 0, 2)
        )

    petab = _pe_table()
    slide = np.zeros((P, 2, 2 * D), f)
    slide[:, :, D - 1] = 1.0

    shared = {
        "wih": lhsT8(beta / SX * Wg_ih.T, 2),
        "whh": lhsT8(beta / 2.0 * Wg_hh.T, 2),
        "w1": lhsT8(gamma / 2.0 * np.asarray(W1, f).T, 2),
        "w2": lhsT8(DELTA / SH * np.asarray(W2, f).T, 4),
        "slide": _q8(slide),
        "ones64": np.ones((D, 1), f),
    }
    samples = np.asarray(input_samples)
    poss = np.asarray(pos_list)
    te_f = np.asarray(token_embed, f)  # [NCL, E]
    in_maps = []
    for c in range(NCORES):
        lo, hi = c * BS, (c + 1) * BS
        sa = samples[lo:hi]  # [BS, D]
        po = poss[lo:hi]
        ohs = np.zeros((D, 2, P, BS), NPF8)
        ii = np.arange(BS)
        for i in range(D):
            s = np.asarray(sa[:, i])
            ohs[i, s // P, s % P, ii] = 1.0
        ohs = np.ascontiguousarray(ohs.transpose(0, 2, 1, 3))
        # full LSTM input per step: x_i = te[s_i] + pe(pos_i), fp8 at 64x
        xpe = _q8(SX * (te_f[sa.T] + petab[po.T]))  # [D, BS, E]
        xpe = np.ascontiguousarray(
            xpe.transpose(0, 2, 1).reshape(D, 2, P, BS).transpose(0, 2, 1, 3)
        )
        x0pe = _q8(SX * petab[po[:, 0]])  # [BS, E] — init cell input (pe only)
        x0pe = np.ascontiguousarray(
            x0pe.T.reshape(2, P, BS).transpose(1, 0, 2)
        )
        m = dict(shared)
        m["ohs"] = ohs
        m["xpe"] = xpe
        m["x0pe"] = x0pe
        in_maps.append(m)
    return in_maps


_CACHE = {}


def kernel(**inputs) -> np.ndarray:
    global _BETA, _GAMMA
    if "nc" not in _CACHE:
        _BETA, _GAMMA, _, _ = _compute_scales(
            inputs["W_ih"], inputs["W_hh"], inputs["W1"]
        )
        _CACHE["nc"] = build_bass()
    nc = _CACHE["nc"]
    in_maps = prep_inputs(**inputs)
    res = bass_utils.run_bass_kernel_spmd(nc, in_maps, core_ids=list(range(NCORES)))
    _CACHE["last_results"] = res
    out = np.empty((B, 1), np.float32)
    for c in range(NCORES):
        out[c * BS : (c + 1) * BS, 0] = np.asarray(
            res.results[c]["out"], np.float32
        ).reshape(BS)
    return out



# revision 7
# speedup vs baseline: 1.0980x; 1.0070x over previous
"""Trainium2 Bass kernel for nn_CondRnnSampler — v3.

v2 (fp8 DoubleRow + all-tanh) was ACT-engine bound at ~9 ACTIVATE ops/step
(~1250 ns each, 95% busy); after rebalancing the kernel is chain-latency
bound (~9.9 us/step), so emission order targets the recurrence path.
v3 changes:
  - x = te[s]+pe gathered fully host-side (fp8 stream) — drops the te
    matmuls and the DVE x-adds.
  - esum/pick selector matmuls run DoubleRow over both k-halves (2 MMs/step
    instead of 4).
  - pick-product reads the logits PSUM directly on DVE (drops the ACT COPY).
  - both MLP relus on DVE, emitted via gate_step's mid_cb so their W1
    matmuls run right after whh-i on PE and the relus clear DVE before the
    chain junction (x2/s') needs it; the o-gate block comes after.
  - cell update as TS (4x) + TT (2x) + STT mix; x1 = sf*s_prev runs on
    GpSimd (off the critical path — the junction is the x2 path).
ACT per step: 4 gate tanh + state tanh + exp = 6 ops.

Per-core (512 rows), per step:
  MLP:   hid = relu(W1 h), logits = W2 hid, e = exp(logits), prod = logits*oh
  cell:  gates = W_ih x + W_hh h (fp8 DoubleRow, K=256/instr), all-tanh via
         sigma(z) = (1+tanh(z/2))/2.  State: s = 2c (bf16), v = 2h (fp8):
           s' = 0.5*(1+tf)*s + (1+ti)*g ;  v' = (1+to)*tanh(0.5 s')
  out:   esum/pick accumulate into [64,BS] PSUM banks via sliding-selector
         fp8 DoubleRow matmuls.

Scales (folded on host): x8 = 64*x, v = 2h, hid8 = 8*hid, gates PSUM = beta*a,
logits PSUM = delta*l.  One-hots (sample) and the full gathered LSTM input
(te[s]+pe) are built host-side and DMA-streamed.
"""

import sys

sys.path.insert(0, "/opt/trn_rl_repo")

from contextlib import ExitStack

import ml_dtypes
import numpy as np

import concourse.bacc as bacc
import concourse.tile as tile
from concourse import bass_utils, mybir
from concourse.bass import ts

B, D, E, NCL = 4096, 64, 256, 256
NCORES = 8
BS = B // NCORES
P = 128

AF = mybir.ActivationFunctionType
OP = mybir.AluOpType
F32 = mybir.dt.float32
BF16 = mybir.dt.bfloat16
FP8 = mybir.dt.float8e4
DR = mybir.MatmulPerfMode.DoubleRow
NPBF = ml_dtypes.bfloat16
NPF8 = ml_dtypes.float8_e4m3

SX = 64.0  # x fp8 scale
SH = 8.0  # hid fp8 scale
DELTA = 256.0  # logits PSUM scale


def _pe_table() -> np.ndarray:
    half = np.float32(E // 2)
    inv = (
        np.float32(1.0)
        / (np.float32(10000.0) ** (np.arange(E // 2, dtype=np.float32) / half))
    ).astype(np.float32)
    pos = np.arange(D, dtype=np.float32)[:, None]
    ang = pos * inv[None, :]
    return np.concatenate([np.sin(ang), np.cos(ang)], axis=1).astype(np.float32)


def _q8(x):
    return np.clip(np.asarray(x, np.float32), -240, 240).astype(NPF8)


def build_bass(n_steps: int = D):
    nc = bacc.Bacc("TRN2", debug=False, target_bir_lowering=False, num_devices=NCORES)

    def din(name, shape, dt):
        return nc.dram_tensor(name, list(shape), dt, kind="ExternalInput").ap()

    wih_d = din("wih", (P, 2, 4 * E), FP8)
    whh_d = din("whh", (P, 2, 4 * E), FP8)
    w1_d = din("w1", (P, 2, 2 * E), FP8)
    w2_d = din("w2", (P, 4, NCL), FP8)
    slide_d = din("slide", (P, 2, 2 * D), FP8)  # ones at col D-1 (both halves)
    ones64_d = din("ones64", (D, 1), F32)
    ohs_d = din("ohs", (D, P, 2, BS), FP8)  # one-hot(sample) per step
    xpe_d = din("xpe", (D, P, 2, BS), FP8)  # 64*(te[s]+petab[pos]) per step
    x0pe_d = din("x0pe", (P, 2, BS), FP8)  # 64*petab[pos_0] (init cell input)
    out_d = nc.dram_tensor("out", [1, BS], F32, kind="ExternalOutput").ap()

    with tile.TileContext(nc) as tc:
        with ExitStack() as ctx:
            sing = ctx.enter_context(tc.tile_pool(name="sing", bufs=1))
            gt = ctx.enter_context(tc.tile_pool(name="gt", bufs=6))
            gt2 = ctx.enter_context(tc.tile_pool(name="gt2", bufs=2))
            hp = ctx.enter_context(tc.tile_pool(name="hp", bufs=3))
            ep = ctx.enter_context(tc.tile_pool(name="ep", bufs=4))
            psing = ctx.enter_context(tc.tile_pool(name="psing", bufs=1, space="PSUM"))
            pp = ctx.enter_context(tc.tile_pool(name="pp", bufs=3, space="PSUM"))

            # ---- resident tensors -------------------------------------
            # init-critical first: step-0 one-hot/pe slices + gate weights
            ohs_sb = sing.tile([P, D, 2, BS], FP8, tag="ohs")
            xpe_sb = sing.tile([P, D, 2, BS], FP8, tag="xpe")
            x0pe_sb = sing.tile([P, 2, BS], FP8, tag="x0pe")
            nc.sync.dma_start(x0pe_sb[:], x0pe_d)
            nc.sync.dma_start(xpe_sb[:, 0], xpe_d[0])
            nc.sync.dma_start(ohs_sb[:, 0], ohs_d[0])
            wih = sing.tile([P, 2, 4 * E], FP8, tag="wih")
            nc.sync.dma_start(wih[:], wih_d)
            whh = sing.tile([P, 2, 4 * E], FP8, tag="whh")
            nc.sync.dma_start(whh[:], whh_d)
            w1 = sing.tile([P, 2, 2 * E], FP8, tag="w1")
            nc.sync.dma_start(w1[:], w1_d)
            w2 = sing.tile([P, 4, NCL], FP8, tag="w2")
            nc.sync.dma_start(w2[:], w2_d)
            slide = sing.tile([P, 2, 2 * D], FP8, tag="slide")
            nc.sync.dma_start(slide[:], slide_d)
            ones64 = sing.tile([D, 1], F32, tag="ones64")
            nc.sync.dma_start(ones64[:], ones64_d)

            for i in range(1, n_steps):
                nc.sync.dma_start(ohs_sb[:, i], ohs_d[i])
                nc.sync.dma_start(xpe_sb[:, i], xpe_d[i])

            # double-buffered recurrent state (parity by step)
            s_bufs = [
                sing.tile([P, 2, BS], BF16, tag=f"s{j}", name=f"s{j}")
                for j in range(2)
            ]
            v_bufs = [
                sing.tile([P, 2, BS], FP8, tag=f"v{j}", name=f"v{j}")
                for j in range(2)
            ]
            T_sb = sing.tile([P, 2, BS], BF16, tag="T")
            esum_ps = psing.tile([D, BS], F32, tag="esum")
            pick_ps = psing.tile([D, BS], F32, tag="pick")

            # scales arrive via sc tile? No - bake as python floats at build:
            # (they depend only on weight maxima; recomputed per call would
            # need rebuild. Instead scales are fixed: beta/gamma baked by
            # prep_inputs to match BETA/GAMMA globals.)

            def gate_step(x8_ap, v_prev, with_h, inv_beta, mid_cb=None):
                """gates -> t tiles [ti, tf, g, to]; order f,g,i,o so the
                chain ops X1 (needs tf) and X2 (needs g) unblock earliest.

                mid_cb (default-priority emissions) runs between the i- and
                o-gate blocks: the W1 matmuls land on PE right after whh-i,
                so the relus become ready (and clear DVE) well before the
                ti1/x2/s' chain junction needs the engine."""
                tg = [None] * 4
                # v-independent wih matmuls for the chain-leading f/g gates
                # are emitted at normal priority AHEAD of any whh matmul, so
                # the in-order PE queue runs them during the v-wait bubble
                # instead of stalling behind the first v-dependent whh.
                pre = {}
                if with_h:
                    for gi in (1, 2, 0):  # f, g, i
                        g_ps = pp.tile([P, 2, BS], F32, tag="ps")
                        for k in range(2):
                            nc.tensor.matmul(
                                g_ps[:, k, :], wih[:, :, ts(gi * 2 + k, P)],
                                x8_ap, start=True, stop=False, perf_mode=DR,
                            )
                        pre[gi] = g_ps
                with tc.high_priority():
                    for gi in (1, 2, 0):  # f, g, i
                        if gi in pre:
                            g_ps = pre[gi]
                            for k in range(2):
                                nc.tensor.matmul(
                                    g_ps[:, k, :], whh[:, :, ts(gi * 2 + k, P)],
                                    v_prev[:], start=False, stop=True,
                                    perf_mode=DR,
                                )
                        else:
                            g_ps = pp.tile([P, 2, BS], F32, tag="ps")
                            for k in range(2):
                                m = gi * 2 + k
                                nc.tensor.matmul(
                                    g_ps[:, k, :], wih[:, :, ts(m, P)], x8_ap,
                                    start=True, stop=not with_h, perf_mode=DR,
                                )
                                if with_h:
                                    nc.tensor.matmul(
                                        g_ps[:, k, :], whh[:, :, ts(m, P)],
                                        v_prev[:], start=False, stop=True,
                                        perf_mode=DR,
                                    )
                        t_sb = gt.tile([P, 2, BS], BF16, tag="t")
                        nc.scalar.activation(
                            t_sb[:], g_ps[:], AF.Tanh, scale=inv_beta
                        )
                        tg[gi] = t_sb
                if mid_cb is not None:
                    mid_cb()
                # o-gate last: its tanh is only needed by v' (after T)
                g_ps = pp.tile([P, 2, BS], F32, tag="ps")
                for k in range(2):
                    m = 3 * 2 + k
                    nc.tensor.matmul(
                        g_ps[:, k, :], wih[:, :, ts(m, P)], x8_ap,
                        start=True, stop=not with_h, perf_mode=DR,
                    )
                    if with_h:
                        nc.tensor.matmul(
                            g_ps[:, k, :], whh[:, :, ts(m, P)],
                            v_prev[:], start=False, stop=True, perf_mode=DR,
                        )
                t_sb = gt.tile([P, 2, BS], BF16, tag="t")
                nc.scalar.activation(t_sb[:], g_ps[:], AF.Tanh, scale=inv_beta)
                tg[3] = t_sb
                return tg

            def tail(tg, s_prev, s_cur, v_cur, first):
                """Recurrent-chain ops at high priority so the scheduler's
                static per-engine orders never park bulk work (relu/prod/
                exp) in front of them.

                Cell update decomposed into TS (4x bf16 mode) + TT (2x)
                ops instead of 1x-only STT:
                  sf  = 0.5 + 0.5*tf          (TS, 4x; folds the 0.5 of s')
                  x1  = sf * s_prev           (TT on GpSimd: tf/s_prev ready
                                               early, keeps DVE free)
                  ti1 = 1 + ti                (TS, 4x)
                  x2  = ti1 * g               (TT, 2x)
                  s'  = x1 + x2               (TT, 2x)
                  v'  = (1+to)*T              (STT: fp8 out is 1x anyway)
                """
                ti, tf, g, to = tg[0], tg[1], tg[2], tg[3]
                with tc.high_priority():
                    if first:
                        # s = (1+ti)*g
                        nc.vector.scalar_tensor_tensor(
                            s_cur[:], ti[:], 1.0, g[:], OP.add, OP.mult
                        )
                    else:
                        # sf = 0.5+0.5*tf (TS, 4x — folds s'-halving); x1 =
                        # sf*s (TT, 2x); x2 via STT (chain junction: one hop
                        # after tanh-i); s' = x1+x2 (TT, 2x)
                        sf = gt2.tile([P, 2, BS], BF16, tag="sf")
                        nc.vector.tensor_scalar(
                            sf[:], tf[:], 0.5, 0.5, OP.mult, OP.add
                        )
                        x1 = gt2.tile([P, 2, BS], BF16, tag="x1")
                        # split across engines: the GpSimd half ends before
                        # the ti1/x2 junction (Pool shares the DVE SBUF port
                        # and a full-size op stretches junction ops ~+500ns)
                        nc.gpsimd.tensor_tensor(
                            x1[:, 0, :], sf[:, 0, :], s_prev[:, 0, :], OP.mult
                        )
                        nc.vector.tensor_tensor(
                            x1[:, 1, :], sf[:, 1, :], s_prev[:, 1, :], OP.mult
                        )
                        ti1 = gt2.tile([P, 2, BS], BF16, tag="ti1")
                        nc.vector.tensor_scalar(
                            ti1[:], ti[:], 1.0, None, OP.add
                        )
                        x2 = gt2.tile([P, 2, BS], BF16, tag="x2")
                        nc.vector.tensor_tensor(x2[:], ti1[:], g[:], OP.mult)
                        nc.vector.tensor_tensor(s_cur[:], x1[:], x2[:], OP.add)
                    nc.scalar.activation(T_sb[:], s_cur[:], AF.Tanh, scale=0.5)
                    nc.vector.scalar_tensor_tensor(
                        v_cur[:], to[:], 1.0, T_sb[:], OP.add, OP.mult
                    )

            inv_beta = float(1.0 / _BETA)
            hid_scale = float(SH / _GAMMA)
            inv_delta = float(1.0 / DELTA)

            # ---- init: lstm(pe_0) with zero state ---------------------
            # the init cell input is pe[pos[:,0]] alone (no token embed) so
            # it gets its own x0pe stream; scan step i uses xpe slot i =
            # te[s_i] + pe_i.
            tg0 = gate_step(x0pe_sb[:], None, with_h=False, inv_beta=inv_beta)
            tail(tg0, None, s_bufs[1], v_bufs[1], first=True)

            pending = []  # deferred (step, e8, pr8) awaiting esum/pick MMs

            def flush_accum(j, e8_j, pr8_j):
                nc.tensor.matmul(
                    esum_ps[:], slide[:, :, D - 1 - j : 2 * D - 1 - j],
                    e8_j[:], start=(j == 0), stop=(j == n_steps - 1),
                    perf_mode=DR, skip_group_check=True,
                )
                nc.tensor.matmul(
                    pick_ps[:], slide[:, :, D - 1 - j : 2 * D - 1 - j],
                    pr8_j[:], start=(j == 0), stop=(j == n_steps - 1),
                    perf_mode=DR, skip_group_check=True,
                )

            # ---- scan -------------------------------------------------
            for i in range(n_steps):
                v_prev, v_cur = v_bufs[(i + 1) % 2], v_bufs[i % 2]
                s_prev, s_cur = s_bufs[(i + 1) % 2], s_bufs[i % 2]

                hid8 = []
                h1_ps = []

                def mlp_front(v_prev=v_prev, hid8=hid8, h1_ps=h1_ps):
                    # W1 matmuls emitted mid-gate-block: PE runs them right
                    # after whh-i.  h0-relu on DVE here (clears the engine
                    # before the x2/s' junction); h1-relu is emitted after
                    # the o-gate so the ACT order is f,g,i,o,relu1,T.
                    for hh in range(2):
                        h_ps = pp.tile([P, 2, BS], F32, tag="ps")
                        for k in range(2):
                            m = hh * 2 + k
                            nc.tensor.matmul(
                                h_ps[:, k, :], w1[:, :, ts(m, P)], v_prev[:],
                                start=True, stop=True, perf_mode=DR,
                            )
                        if hh == 0:
                            h8 = hp.tile([P, 2, BS], FP8, tag="h8")
                            nc.vector.tensor_scalar(
                                h8[:], h_ps[:], hid_scale, 0.0, OP.mult, OP.max
                            )
                            hid8.append(h8)
                        else:
                            h1_ps.append(h_ps)

                # gates + cell update FIRST (the serial chain)
                tg = gate_step(
                    xpe_sb[:, i], v_prev, with_h=True, inv_beta=inv_beta,
                    mid_cb=mlp_front,
                )
                h8b = hp.tile([P, 2, BS], FP8, tag="h8")
                nc.scalar.activation(
                    h8b[:], h1_ps[0][:], AF.Relu, scale=hid_scale
                )
                hid8.append(h8b)
                tail(tg, s_prev, s_cur, v_cur, first=False)

                l_ps = pp.tile([P, 2, BS], F32, tag="ps")
                for t in range(2):
                    for j in range(2):
                        nc.tensor.matmul(
                            l_ps[:, t, :], w2[:, 2 * j : 2 * j + 2, ts(t, P)],
                            hid8[j][:], start=(j == 0), stop=(j == 1),
                            perf_mode=DR,
                        )
                e8 = ep.tile([P, 2, BS], FP8, tag="e8")
                nc.scalar.activation(e8[:], l_ps[:], AF.Exp, scale=inv_delta)
                # pick-product straight from the logits PSUM on DVE (one op;
                # the l_ps banks free after this + the exp read)
                pr8 = ep.tile([P, 2, BS], FP8, tag="pr8")
                nc.vector.tensor_tensor(
                    pr8[:], l_ps[:], ohs_sb[:, i], OP.mult
                )

                # esum/pick accumulation (fp8 DoubleRow; M=64 dst), deferred
                # by one step so these MMs never sit in the PE's in-order
                # queue ahead of the next step's chain-critical gate matmuls
                # while still waiting on exp/prod outputs.
                pending.append((i, e8, pr8))
                if i > 0:
                    flush_accum(*pending.pop(0))

            # ---- epilogue ---------------------------------------------
            while pending:
                flush_accum(*pending.pop(0))
            ln_e = sing.tile([D, BS], F32, tag="lne")
            nc.scalar.activation(ln_e[:], esum_ps[:], AF.Ln)
            diff = sing.tile([D, BS], F32, tag="diff")
            nc.vector.scalar_tensor_tensor(
                diff[:], pick_ps[:], inv_delta, ln_e[:],
                OP.mult, OP.subtract,
            )
            fin_ps = pp.tile([P, 2, BS], F32, tag="ps")
            nc.tensor.matmul(
                fin_ps[0:1, 0, :], ones64[:, 0:1], diff[:], start=True, stop=True
            )
            out_sb = sing.tile([1, BS], F32, tag="outsb")
            nc.scalar.activation(out_sb[:], fin_ps[0:1, 0, :], AF.Copy)
            nc.sync.dma_start(out_d, out_sb[:])

    nc.compile()
    return nc


_BETA = None
_GAMMA = None


def _compute_scales(W_ih, W_hh, W1):
    half = np.ones((4 * E, 1), np.float32)
    half[: 2 * E] = 0.5
    half[3 * E :] = 0.5
    Wg_ih = np.asarray(W_ih, np.float32) * half
    Wg_hh = np.asarray(W_hh, np.float32) * half
    beta = 216.0 / max(np.abs(Wg_ih / SX).max(), np.abs(Wg_hh / 2.0).max())
    gamma = 216.0 / np.abs(np.asarray(W1, np.float32) / 2.0).max()
    return beta, gamma, Wg_ih, Wg_hh


def prep_inputs(token_embed, W_ih, b_ih, b_hh, W_hh, W1, b1, W2, b2, pos_list,
                input_samples):
    f = np.float32
    for b in (b_ih, b_hh, b1, b2):
        assert np.all(np.asarray(b) == 0), "nonzero biases unsupported"
    beta, gamma, Wg_ih, Wg_hh = _compute_scales(W_ih, W_hh, W1)
    assert beta == _BETA and gamma == _GAMMA

    def lhsT8(Wt, ko):  # [K, M] -> [P, ko, M] fp8
        K, M = Wt.shape
        return np.ascontiguousarray(
            _q8(Wt).reshape(ko, P, M).transpose(1, 0, 2)
        )

    petab = _pe_table()
    slide = np.zeros((P, 2, 2 * D), f)
    slide[:, :, D - 1] = 1.0

    shared = {
        "wih": lhsT8(beta / SX * Wg_ih.T, 2),
        "whh": lhsT8(beta / 2.0 * Wg_hh.T, 2),
        "w1": lhsT8(gamma / 2.0 * np.asarray(W1, f).T, 2),
        "w2": lhsT8(DELTA / SH * np.asarray(W2, f).T, 4),
        "slide": _q8(slide),
        "ones64": np.ones((D, 1), f),
    }
    samples = np.asarray(input_samples)
    poss = np.asarray(pos_list)
    te_f = np.asarray(token_embed, f)  # [NCL, E]
    in_maps = []
    for c in range(NCORES):
        lo, hi = c * BS, (c + 1) * BS
        sa = samples[lo:hi]  # [BS, D]
        po = poss[lo:hi]
        ohs = np.zeros((D, 2, P, BS), NPF8)
        ii = np.arange(BS)
        for i in range(D):
            s = np.asarray(sa[:, i])
            ohs[i, s // P, s % P, ii] = 1.0
        ohs = np.ascontiguousarray(ohs.transpose(0, 2, 1, 3))
        # full LSTM input per step: x_i = te[s_i] + pe(pos_i), fp8 at 64x
        xpe = _q8(SX * (te_f[sa.T] + petab[po.T]))  # [D, BS, E]
        xpe = np.ascontiguousarray(
            xpe.transpose(0, 2, 1).reshape(D, 2, P, BS).transpose(0, 2, 1, 3)
        )
        x0pe = _q8(SX * petab[po[:, 0]])  # [BS, E] — init cell input (pe only)
        x0pe = np.ascontiguousarray(
            x0pe.T.reshape(2, P, BS).transpose(1, 0, 2)
        )
        m = dict(shared)
        m["ohs"] = ohs
        m["xpe"] = xpe
        m["x0pe"] = x0pe
        in_maps.append(m)
    return in_maps


_CACHE = {}


def kernel(**inputs) -> np.ndarray:
    global _BETA, _GAMMA
    if "nc" not in _CACHE:
        _BETA, _GAMMA, _, _ = _compute_scales(
            inputs["W_ih"], inputs["W_hh"], inputs["W1"]
        )
        _CACHE["nc"] = build_bass()
    nc = _CACHE["nc"]
    in_maps = prep_inputs(**inputs)
    res = bass_utils.run_bass_kernel_spmd(nc, in_maps, core_ids=list(range(NCORES)))
    _CACHE["last_results"] = res
    out = np.empty((B, 1), np.float32)
    for c in range(NCORES):
        out[c * BS : (c + 1) * BS, 0] = np.asarray(
            res.results[c]["out"], np.float32
        ).reshape(BS)
    return out

